# revision 11
# baseline (speedup 1.0000x reference)
"""DeepGCN ResNet (4-layer GCN w/ residuals + log_softmax) on 8 TRN2 NeuronCores.

Sharding: nodes (rows) split 8 ways; edges partitioned by destination row.
Per layer: local dense x@W -> AllGather h -> dma_gather of source rows ->
one-hot segment-matmul on TensorE (PSUM accumulate per 128-dest window) ->
+res0, relu. Final layer + res0@W_res_last + log_softmax.

Host-side preprocessing only does layout work: edge sorting by (dest window,
src half), padding to a core-uniform tile structure, one-hot tile
construction, and int16 gather-index packing (sources split in two halves so
row indices fit int16).
"""

import math

import numpy as np

import concourse.bacc as bacc
import concourse.bass as bass
import concourse.mybir as mybir
import concourse.tile as tile
from concourse.bass_utils import run_bass_kernel_spmd

P = 128

# Problem geometry (hardcoded per the task contract).
N_NODES = 50000
N_EDGES = 800000
F_IN = 256
HID = 256
C_OUT = 40
NCORES = 8

NLOC = N_NODES // NCORES            # 6250
NPAD = ((NLOC + P - 1) // P) * P    # 6272
W_WIN = NPAD // P                   # 49 windows of 128 dest rows
HALFROWS = (NCORES // 2) * NPAD     # 25088 (< int16 max) split of h_full rows


def _set_geometry(n_nodes):
    """Debug hook: shrink the node count (keeps F/HID/C). Used only by the
    small-scale simulator test, never in grading."""
    global N_NODES, NLOC, NPAD, W_WIN, HALFROWS
    N_NODES = n_nodes
    NLOC = N_NODES // NCORES
    NPAD = ((NLOC + P - 1) // P) * P
    W_WIN = NPAD // P
    HALFROWS = (NCORES // 2) * NPAD

F32 = mybir.dt.float32
BF16 = mybir.dt.bfloat16
I16 = mybir.dt.int16
MAX_GATHER_TILES = 999
GATHER_SINGLE_PACKET = False


# ----------------------------------------------------------------- host prep


def preprocess(edge_row, edge_col, edge_val):
    """Edge partitioning/sorting/padding. Returns per-core data arrays plus a
    core-uniform tile structure (identical program for all cores)."""
    edge_row = edge_row.astype(np.int64)
    edge_col = edge_col.astype(np.int64)
    core = edge_row // NLOC
    d_local = edge_row - core * NLOC
    win = d_local // P
    d8 = d_local % P
    src_core = edge_col // NLOC
    srcg = src_core * NPAD + (edge_col - src_core * NLOC)
    is_hi = (srcg >= HALFROWS).astype(np.int64)

    lo_cnt = np.zeros((NCORES, W_WIN), np.int64)
    hi_cnt = np.zeros((NCORES, W_WIN), np.int64)
    np.add.at(lo_cnt, (core, win), 1 - is_hi)
    np.add.at(hi_cnt, (core, win), is_hi)

    lo_T = np.maximum(1, -(-lo_cnt.max(axis=0) // P))
    hi_T = -(-hi_cnt.max(axis=0) // P)
    win_T = lo_T + hi_T
    T_total = int(win_T.sum())

    lo_base = np.zeros(W_WIN, np.int64)
    hi_base = np.zeros(W_WIN, np.int64)
    runs = []  # per window: [(tile_offset, n_tiles, is_hi), ...]
    t = 0
    for w in range(W_WIN):
        lo_base[w] = t * P
        wruns = [(t, int(lo_T[w]), 0)]
        t += int(lo_T[w])
        hi_base[w] = t * P
        if hi_T[w] > 0:
            wruns.append((t, int(hi_T[w]), 1))
        t += int(hi_T[w])
        runs.append(wruns)
    assert t == T_total

    idx_all = np.zeros((NCORES, T_total * P), np.int16)
    oh_val = np.zeros((NCORES, T_total * P), np.float32)
    oh_d8 = np.zeros((NCORES, T_total * P), np.int64)

    order = np.lexsort((is_hi, win, core))
    e_core, e_win, e_hi = core[order], win[order], is_hi[order]
    e_srcg, e_d8, e_val = srcg[order], d8[order], edge_val[order]

    grp = (e_core * W_WIN + e_win) * 2 + e_hi
    cnt = np.zeros(NCORES * W_WIN * 2 + 1, np.int64)
    np.add.at(cnt, grp + 1, 1)
    starts = np.cumsum(cnt)[:-1]
    within = np.arange(len(order)) - starts[grp]

    slot = np.where(e_hi == 0, lo_base[e_win], hi_base[e_win]) + within
    idx_all[e_core, slot] = (e_srcg - e_hi * HALFROWS).astype(np.int16)
    oh_val[e_core, slot] = e_val
    oh_d8[e_core, slot] = e_d8

    return dict(idx_all=idx_all, oh_val=oh_val, oh_d8=oh_d8, runs=runs,
                win_T=win_T, T_total=T_total)


def build_in_maps(inputs, pp):
    raw_x = np.ascontiguousarray(inputs["raw_x"], dtype=np.float32)
    T = pp["T_total"]

    def wblocks(w, fout_pad):
        wp = np.zeros((w.shape[0], fout_pad), np.float32)
        wp[:, :w.shape[1]] = w
        kh = w.shape[0] // P
        return wp.reshape(kh, P, fout_pad)

    W012 = np.stack([
        np.stack([np.stack([inputs[k][a * P:(a + 1) * P, b * P:(b + 1) * P]
                            for b in range(2)]) for a in range(2)])
        for k in ("W_gc0", "W_gc1", "W_gc2")]).astype(np.float32)
    W3 = wblocks(np.asarray(inputs["W_gc3"], np.float32), 128)
    Wr0 = np.stack([np.stack([inputs["W_res0"][a * P:(a + 1) * P, b * P:(b + 1) * P]
                              for b in range(2)]) for a in range(2)]).astype(np.float32)
    WrL = wblocks(np.asarray(inputs["W_res_last"], np.float32), 64)
    ident = np.eye(P, dtype=np.float32)

    in_maps = []
    for c in range(NCORES):
        xT0 = np.zeros((2, P, NPAD), np.float32)
        xloc = raw_x[c * NLOC:(c + 1) * NLOC]          # [NLOC, 256]
        xT0[:, :, :NLOC] = xloc.T.reshape(2, P, NLOC)

        idx16 = np.zeros((16, T * 8), np.int16)
        i = np.arange(T * P)
        idx16[i % 16, i // 16] = pp["idx_all"][c]
        gidx = np.tile(idx16, (8, 1))                  # [128, T*8]

        import ml_dtypes
        oh = np.zeros((T * P, P), np.float32)
        oh[np.arange(T * P), pp["oh_d8"][c]] = pp["oh_val"][c]
        oh = np.ascontiguousarray(
            oh.reshape(T, P, P).transpose(1, 0, 2)).astype(ml_dtypes.bfloat16)

        in_maps.append(dict(xT0=xT0, gidx=gidx, oh=oh, W012=W012, W3=W3,
                            Wr0=Wr0, WrL=WrL, ident=ident))
    return in_maps


# ------------------------------------------------------------- device program


def build_program(pp):
    T = pp["T_total"]
    win_T = pp["win_T"]
    runs = pp["runs"]
    Tmax = int(win_T.max())
    NG = [(g * 512, min(512, NPAD - g * 512)) for g in range(-(-NPAD // 512))]

    nc = bacc.Bacc("TRN2", target_bir_lowering=False, debug=False,
                   num_devices=NCORES)

    xT0_d = nc.dram_tensor("xT0", [2, P, NPAD], F32, kind="ExternalInput")
    gidx_d = nc.dram_tensor("gidx", [P, T * 8], I16, kind="ExternalInput")
    oh_d = nc.dram_tensor("oh", [P, T, P], BF16, kind="ExternalInput")
    W012_d = nc.dram_tensor("W012", [3, 2, 2, P, P], F32, kind="ExternalInput")
    W3_d = nc.dram_tensor("W3", [2, P, 128], F32, kind="ExternalInput")
    Wr0_d = nc.dram_tensor("Wr0", [2, 2, P, P], F32, kind="ExternalInput")
    WrL_d = nc.dram_tensor("WrL", [2, P, 64], F32, kind="ExternalInput")
    ident_d = nc.dram_tensor("ident", [P, P], F32, kind="ExternalInput")
    out_d = nc.dram_tensor("out", [NPAD, C_OUT], F32, kind="ExternalOutput")

    res0_d = nc.dram_tensor("res0_d", [NPAD, HID], F32)
    rlast_d = nc.dram_tensor("rlast_d", [NPAD, 64], F32)
    ag_in = [nc.dram_tensor(f"ag_in{i}", [NPAD, HID], BF16) for i in range(2)]
    hfull = [nc.dram_tensor(f"hfull{i}", [NCORES * NPAD, HID], BF16,
                            addr_space="Shared") for i in range(2)]
    ag3_in = nc.dram_tensor("ag3_in", [NPAD, 128], BF16)
    h3full = nc.dram_tensor("h3full", [NCORES * NPAD, 128], BF16,
                            addr_space="Shared")

    with tile.TileContext(nc) as tc:
        with (
            tc.tile_pool(name="pers", bufs=1) as pers,
            tc.tile_pool(name="wbufp", bufs=2) as wbufp,
            tc.tile_pool(name="ohp", bufs=2) as ohp,
            tc.tile_pool(name="hTs", bufs=3) as hTsp,
            tc.tile_pool(name="hns", bufs=3) as hnsp,
            tc.tile_pool(name="zs", bufs=3) as zsp,
            tc.tile_pool(name="r0s", bufs=3) as r0sp,
            tc.tile_pool(name="sm", bufs=4) as smp,
            tc.tile_pool(name="ps_s", bufs=2, space="PSUM") as ps_s,
            tc.tile_pool(name="ps_d", bufs=2, space="PSUM") as ps_d,
            tc.tile_pool(name="ps_t", bufs=2, space="PSUM") as ps_t,
        ):
            # ---------------- persistent loads
            xT = pers.tile([P, 2, NPAD], F32, tag="xT")
            nc.sync.dma_start(out=xT[:, 0, :], in_=xT0_d.ap()[0])
            nc.sync.dma_start(out=xT[:, 1, :], in_=xT0_d.ap()[1])
            idxs = pers.tile([P, T * 8], I16, tag="gidx")
            nc.sync.dma_start(out=idxs[:], in_=gidx_d.ap())
            w012 = pers.tile([P, 12, P], F32, tag="w012")
            nc.sync.dma_start(
                out=w012[:],
                in_=W012_d.ap().rearrange("a b c p m -> p (a b c) m"))
            w3 = pers.tile([P, 2, 128], F32, tag="w3")
            nc.sync.dma_start(out=w3[:], in_=W3_d.ap().rearrange("a p m -> p a m"))
            wr0 = pers.tile([P, 4, P], F32, tag="wr0")
            nc.sync.dma_start(out=wr0[:],
                              in_=Wr0_d.ap().rearrange("a b p m -> p (a b) m"))
            wrl = pers.tile([P, 2, 64], F32, tag="wrl")
            nc.sync.dma_start(out=wrl[:], in_=WrL_d.ap().rearrange("a p m -> p a m"))
            ident = pers.tile([P, P], F32, tag="ident")
            nc.sync.dma_start(out=ident[:], in_=ident_d.ap())

            def w012_ap(li, kh, fh):
                return w012[:, li * 4 + kh * 2 + fh, :]

            # ------------- dense helper: h[:, fh*128:...] = x @ W  (+ hooks)
            def dense256(w_ap_fn, dest_dram, rl_hook=None, hn_dt=BF16,
                         hn_tag="hn"):
                """w_ap_fn(kh, fh) -> lhsT [128,128]. Writes node-major
                [NPAD, 256] to dest_dram. rl_hook(fh, g0, ng, hT_stage)."""
                for g0, ng in NG:
                    stages = []
                    for fh in range(2):
                        psd = ps_d.tile([P, 512], F32, space="PSUM", tag="dps")
                        for kh in range(2):
                            nc.tensor.matmul(
                                psd[:, :ng], lhsT=w_ap_fn(kh, fh),
                                rhs=xT[:, kh, g0:g0 + ng],
                                start=(kh == 0), stop=(kh == 1))
                        hTst = hTsp.tile([P, 512], F32, tag="hT")
                        nc.scalar.copy(out=hTst[:, :ng], in_=psd[:, :ng])
                        stages.append(hTst)
                        if rl_hook is not None:
                            rl_hook(fh, g0, ng, hTst)
                    for s in range(ng // P):
                        hn = hnsp.tile([P, HID], hn_dt, tag=hn_tag)
                        for fh in range(2):
                            pst = ps_t.tile([P, P], F32, space="PSUM", tag="tp")
                            nc.tensor.transpose(
                                out=pst[:],
                                in_=stages[fh][:, s * P:(s + 1) * P],
                                identity=ident[:])
                            nc.vector.tensor_copy(
                                out=hn[:, fh * P:(fh + 1) * P], in_=pst[:])
                        nc.sync.dma_start(
                            out=dest_dram.ap()[g0 + s * P:g0 + (s + 1) * P, :],
                            in_=hn[:])

            # ---------------- init: res0 (+ rlast) from raw_x
            rl_ps = {}

            def rl_hook(fh, g0, ng, hTst):
                if fh == 0:
                    rl_ps["t"] = ps_d.tile([P, 512], F32, space="PSUM",
                                           tag="dps", name="psr")
                psr = rl_ps["t"]
                nc.tensor.matmul(psr[:64, :ng], lhsT=wrl[:, fh, :],
                                 rhs=hTst[:, :ng],
                                 start=(fh == 0), stop=(fh == 1))
                if fh == 1:
                    rlT = hTsp.tile([P, 512], F32, tag="hT")
                    nc.scalar.copy(out=rlT[:64, :ng], in_=psr[:64, :ng])
                    for s in range(ng // P):
                        pst = ps_t.tile([P, P], F32, space="PSUM", tag="tp")
                        nc.tensor.transpose(out=pst[:, :64],
                                            in_=rlT[:64, s * P:(s + 1) * P],
                                            identity=ident[:64, :64])
                        rln = hnsp.tile([P, 64], F32, tag="rln")
                        nc.vector.tensor_copy(out=rln[:], in_=pst[:, :64])
                        nc.sync.dma_start(
                            out=rlast_d.ap()[g0 + s * P:g0 + (s + 1) * P, :],
                            in_=rln[:])

            dense256(lambda kh, fh: wr0[:, kh * 2 + fh, :], res0_d,
                     rl_hook=rl_hook, hn_dt=F32, hn_tag="hnf")

            # ---------------- spmm helper
            def spmm(hfull_t, elem, psw, evict_fn):
                pos = 0
                for w in range(W_WIN):
                    nT = int(win_T[w])
                    ohb = ohp.tile([P, Tmax, P], BF16, tag="ohb")
                    nc.sync.dma_start(out=ohb[:, :nT, :],
                                      in_=oh_d.ap()[:, pos:pos + nT, :])
                    wb = wbufp.tile([P, Tmax, elem], BF16, tag="wb")
                    for (t0, nt, hi) in runs[w]:
                        src = (hfull_t.ap()[HALFROWS:, :] if hi
                               else hfull_t.ap()[:HALFROWS, :])
                        for c0 in range(0, nt, MAX_GATHER_TILES):
                            cn = min(MAX_GATHER_TILES, nt - c0)
                            toff = t0 - pos + c0
                            tg = t0 + c0
                            nc.gpsimd.dma_gather(
                                wb[:, toff:toff + cn, :], src,
                                idxs[:, tg * 8:(tg + cn) * 8],
                                cn * P, cn * P, elem,
                                single_packet=GATHER_SINGLE_PACKET)
                    pss = ps_s.tile([P, HID], F32, space="PSUM", tag="sps")
                    for t in range(nT):
                        nc.tensor.matmul(pss[:, :psw],
                                         lhsT=ohb[:, t, :],
                                         rhs=wb[:, t, :],
                                         start=(t == 0), stop=(t == nT - 1))
                    evict_fn(w, pss)
                    pos += nT

            # ---------------- GCN layers 0..2
            for li in range(3):
                dense256(lambda kh, fh, li=li: w012_ap(li, kh, fh), ag_in[li % 2])
                nc.gpsimd.collective_compute(
                    "AllGather", mybir.AluOpType.bypass,
                    replica_groups=[list(range(NCORES))],
                    ins=[ag_in[li % 2].ap()], outs=[hfull[li % 2].ap()])

                def evict_gc(w, pss):
                    r0 = r0sp.tile([P, HID], F32, tag="r0")
                    nc.sync.dma_start(out=r0[:],
                                      in_=res0_d.ap()[w * P:(w + 1) * P, :])
                    z = zsp.tile([P, HID], F32, tag="z")
                    nc.vector.tensor_add(out=z[:], in0=pss[:], in1=r0[:])
                    for fh in range(2):
                        pst = ps_t.tile([P, P], F32, space="PSUM", tag="tp")
                        nc.tensor.transpose(out=pst[:],
                                            in_=z[:, fh * P:(fh + 1) * P],
                                            identity=ident[:])
                        nc.scalar.activation(
                            out=xT[:, fh, w * P:(w + 1) * P], in_=pst[:],
                            func=mybir.ActivationFunctionType.Relu)

                spmm(hfull[li % 2], HID, HID, evict_gc)

            # ---------------- layer 3 dense (fout=64 incl. padding)
            for g0, ng in NG:
                psd = ps_d.tile([P, 512], F32, space="PSUM", tag="dps")
                for kh in range(2):
                    nc.tensor.matmul(psd[:, :ng], lhsT=w3[:, kh, :],
                                     rhs=xT[:, kh, g0:g0 + ng],
                                     start=(kh == 0), stop=(kh == 1))
                h3T = hTsp.tile([P, 512], F32, tag="hT")
                nc.scalar.copy(out=h3T[:, :ng], in_=psd[:, :ng])
                for s in range(ng // P):
                    pst = ps_t.tile([P, P], F32, space="PSUM", tag="tp")
                    nc.tensor.transpose(out=pst[:],
                                        in_=h3T[:, s * P:(s + 1) * P],
                                        identity=ident[:])
                    h3n = hnsp.tile([P, 128], BF16, tag="h3n")
                    nc.vector.tensor_copy(out=h3n[:], in_=pst[:])
                    nc.sync.dma_start(
                        out=ag3_in.ap()[g0 + s * P:g0 + (s + 1) * P, :],
                        in_=h3n[:])
            nc.gpsimd.collective_compute(
                "AllGather", mybir.AluOpType.bypass,
                replica_groups=[list(range(NCORES))],
                ins=[ag3_in.ap()], outs=[h3full.ap()])

            # ---------------- layer 3 spmm + rlast + log_softmax
            def evict_out(w, pss):
                rl = r0sp.tile([P, 64], F32, tag="rl")
                nc.sync.dma_start(out=rl[:],
                                  in_=rlast_d.ap()[w * P:(w + 1) * P, :])
                y = zsp.tile([P, 64], F32, tag="y")
                nc.vector.tensor_add(out=y[:], in0=pss[:, :64], in1=rl[:])
                m = smp.tile([P, 1], F32, tag="m")
                nc.vector.tensor_reduce(out=m[:], in_=y[:, :C_OUT],
                                        axis=mybir.AxisListType.X,
                                        op=mybir.AluOpType.max)
                tt = smp.tile([P, C_OUT], F32, tag="tt")
                nc.vector.tensor_scalar(out=tt[:], in0=y[:, :C_OUT],
                                        scalar1=m[:], scalar2=None,
                                        op0=mybir.AluOpType.subtract)
                e = smp.tile([P, C_OUT], F32, tag="e")
                ssum = smp.tile([P, 1], F32, tag="ss")
                nc.scalar.activation(out=e[:], in_=tt[:],
                                     func=mybir.ActivationFunctionType.Exp,
                                     accum_out=ssum[:])
                lg = smp.tile([P, 1], F32, tag="lg")
                nc.scalar.activation(out=lg[:], in_=ssum[:],
                                     func=mybir.ActivationFunctionType.Ln)
                o = smp.tile([P, C_OUT], F32, tag="o")
                nc.vector.tensor_scalar(out=o[:], in0=tt[:], scalar1=lg[:],
                                        scalar2=None,
                                        op0=mybir.AluOpType.subtract)
                nc.sync.dma_start(out=out_d.ap()[w * P:(w + 1) * P, :],
                                  in_=o[:])

            spmm(h3full, 128, 128, evict_out)

    nc.compile()
    return nc


# ------------------------------------------------------------ timed benchmark


def bench(nc, in_maps, iters=8):
    """Times repeated on-device executions with device-resident inputs
    (replicates bass2jax.run_bass_via_pjrt's multi-core path). Returns
    (best_seconds, per_iter_seconds)."""
    import time

    import jax
    from jax.experimental.shard_map import shard_map
    from jax.sharding import Mesh, NamedSharding, PartitionSpec

    from concourse import bass2jax, mybir as mb

    bass2jax.install_neuronx_cc_hook()

    partition_name = (nc.partition_id_tensor.name
                      if nc.partition_id_tensor else None)
    in_names, out_names, out_avals, zero_outs = [], [], [], []
    for alloc in nc.m.functions[0].allocations:
        if not isinstance(alloc, mb.MemoryLocationSet):
            continue
        name = alloc.memorylocations[0].name
        if alloc.kind == "ExternalInput":
            if name != partition_name:
                in_names.append(name)
        elif alloc.kind == "ExternalOutput":
            out_names.append(name)
            shape = tuple(alloc.tensor_shape)
            dtype = mb.dt.np(alloc.dtype)
            out_avals.append(jax.core.ShapedArray(shape, dtype))
            zero_outs.append(np.zeros(shape, dtype))
    n_params = len(in_names)
    n_outs = len(out_avals)
    all_names = in_names + out_names
    if partition_name is not None:
        all_names = all_names + [partition_name]

    def _body(*args):
        operands = list(args)
        if partition_name is not None:
            operands.append(bass2jax.partition_id_tensor())
        outs = bass2jax._bass_exec_p.bind(
            *operands, out_avals=tuple(out_avals), in_names=tuple(all_names),
            out_names=tuple(out_names), lowering_input_output_aliases=(),
            sim_require_finite=True, sim_require_nnan=True, nc=nc)
        return tuple(outs)

    devices = jax.devices()[:NCORES]
    mesh = Mesh(np.asarray(devices), ("core",))
    in_specs = (PartitionSpec("core"),) * (n_params + n_outs)
    out_specs = (PartitionSpec("core"),) * n_outs
    donate = tuple(range(n_params, n_params + n_outs))
    sharded = jax.jit(shard_map(_body, mesh=mesh, in_specs=in_specs,
                                out_specs=out_specs, check_rep=False),
                      donate_argnums=donate, keep_unused=True)

    sh = NamedSharding(mesh, PartitionSpec("core"))
    dev_in = [
        jax.device_put(
            np.concatenate([np.asarray(in_maps[c][n]) for c in range(NCORES)],
                           axis=0), sh)
        for n in in_names]
    zglobal = [np.zeros((NCORES * z.shape[0], *z.shape[1:]), z.dtype)
               for z in zero_outs]

    times = []
    for _ in range(iters):
        dz = [jax.device_put(z, sh) for z in zglobal]
        for d in dz:
            d.block_until_ready()
        t0 = time.perf_counter()
        outs = sharded(*dev_in, *dz)
        for o in outs:
            o.block_until_ready()
        times.append(time.perf_counter() - t0)
    return min(times), times


# ---------------------------------------------------------------- entry point

_CACHE = {}


def _run(inputs, trace=False, trace_kwargs=None):
    pp = preprocess(np.asarray(inputs["edge_row"]),
                    np.asarray(inputs["edge_col"]),
                    np.asarray(inputs["edge_val"], dtype=np.float32))
    in_maps = build_in_maps(inputs, pp)
    key = "prog"
    if key not in _CACHE:
        _CACHE[key] = build_program(pp)
    nc = _CACHE[key]
    res = run_bass_kernel_spmd(nc, in_maps, list(range(NCORES)), trace=trace,
                               **(trace_kwargs or {}))
    outs = [res.results[c]["out"][:NLOC] for c in range(NCORES)]
    full = np.concatenate(outs, axis=0).astype(np.float32)
    return full, res


def kernel(**inputs):
    out, _ = _run(inputs)
    return out


# revision 20
# speedup vs baseline: 11.0429x; 11.0429x over previous
"""DeepGCN ResNet (4-layer GCN w/ residuals + log_softmax) on 8 TRN2 NeuronCores.

Sharding: nodes (rows) split 8 ways; edges partitioned by destination row.
Per layer: local dense x@W -> AllGather h -> dma_gather of source rows ->
one-hot segment-matmul on TensorE (PSUM accumulate per 128-dest window) ->
+res0, relu. Final layer + res0@W_res_last + log_softmax.

Host-side preprocessing only does layout work: edge sorting by (dest window,
src half), padding to a core-uniform tile structure, one-hot tile
construction, and int16 gather-index packing (sources split in two halves so
row indices fit int16).
"""

import numpy as np

import concourse.bacc as bacc
import concourse.mybir as mybir
import concourse.tile as tile
from concourse.bass_utils import run_bass_kernel_spmd

P = 128

# Problem geometry (hardcoded per the task contract).
N_NODES = 50000
N_EDGES = 800000
F_IN = 256
HID = 256
C_OUT = 40
NCORES = 8

NLOC = N_NODES // NCORES            # 6250
NPAD = ((NLOC + P - 1) // P) * P    # 6272
W_WIN = NPAD // P                   # 49 windows of 128 dest rows
HALFROWS = (NCORES // 2) * NPAD     # 25088 (< int16 max) split of h_full rows


def _set_geometry(n_nodes):
    """Debug hook: shrink the node count (keeps F/HID/C). Used only by the
    small-scale simulator test, never in grading."""
    global N_NODES, NLOC, NPAD, W_WIN, HALFROWS
    N_NODES = n_nodes
    NLOC = N_NODES // NCORES
    NPAD = ((NLOC + P - 1) // P) * P
    W_WIN = NPAD // P
    HALFROWS = (NCORES // 2) * NPAD

F32 = mybir.dt.float32
BF16 = mybir.dt.bfloat16
I16 = mybir.dt.int16
MAX_GATHER_TILES = 999
GATHER_SINGLE_PACKET = False
SKIP_GATHER = False      # timing bisect: omit dma_gather calls
SKIP_SPMM_MM = False     # timing bisect: omit segment matmuls
GATHER_QUEUES = 1        # rotate dma_gather queue_num over this many queues


# ----------------------------------------------------------------- host prep


def preprocess(edge_row, edge_col, edge_val):
    """Edge partitioning/sorting/padding with per-core dest->window
    rebalancing (greedy 2D bin-pack on lo/hi in-degree). Returns per-core
    data arrays plus a core-uniform tile structure."""
    edge_row = edge_row.astype(np.int64)
    edge_col = edge_col.astype(np.int64)

    # --- lo/hi in-degree per destination (lo = src owned by cores 0..3)
    src_core0 = edge_col // NLOC
    src_hi0 = src_core0 >= (NCORES // 2)
    lo_deg = np.zeros(N_NODES, np.int64)
    hi_deg = np.zeros(N_NODES, np.int64)
    np.add.at(lo_deg, edge_row, ~src_hi0)
    np.add.at(hi_deg, edge_row, src_hi0)

    # --- per-core greedy assignment of dests to windows (balance lo & hi)
    pos_of_node = np.zeros(N_NODES, np.int64)
    perms = []  # per core: padded_pos[d_local]
    for c in range(NCORES):
        ld = lo_deg[c * NLOC:(c + 1) * NLOC].astype(np.float64)
        hd = hi_deg[c * NLOC:(c + 1) * NLOC].astype(np.float64)
        order = np.argsort(-(ld + hd), kind="stable")
        # windows 0..N_BIG-1 get a 9-tile budget (1152); rest hard-capped at
        # 1024 so they stay 8 tiles. Every core's excess lands in the same
        # window indices, so the cross-core max stays tight.
        N_BIG = 5
        cap = np.full(W_WIN, 8.0 * P)
        cap[:N_BIG] = 9.0 * P
        loads_lo = np.zeros(W_WIN)
        loads_hi = np.zeros(W_WIN)
        counts = np.zeros(W_WIN, np.int64)
        wassign = np.zeros(NLOC, np.int64)
        for t, d in enumerate(order):
            ccap = min(P, t // W_WIN + 2)  # stay within 2 of even fill
            cost = np.maximum(loads_lo + ld[d], loads_hi + hd[d])
            infeas = ((counts >= ccap) | (loads_lo + ld[d] > cap)
                      | (loads_hi + hd[d] > cap))
            if infeas.all():
                infeas = counts >= ccap
            if infeas.all():
                infeas = counts >= P
            cost = np.where(infeas, 1e18, cost)
            w = int(np.argmin(cost))
            wassign[d] = w
            loads_lo[w] += ld[d]
            loads_hi[w] += hd[d]
            counts[w] += 1
        # slot within window
        slot_in_w = np.zeros(NLOC, np.int64)
        fill = np.zeros(W_WIN, np.int64)
        for d in range(NLOC):
            w = wassign[d]
            slot_in_w[d] = fill[w]
            fill[w] += 1
        p = wassign * P + slot_in_w
        perms.append(p)
        pos_of_node[c * NLOC:(c + 1) * NLOC] = c * NPAD + p

    core = edge_row // NLOC
    p_local = pos_of_node[edge_row] - core * NPAD
    win = p_local // P
    d8 = p_local % P
    srcg = pos_of_node[edge_col]
    is_hi = (srcg >= HALFROWS).astype(np.int64)

    lo_cnt = np.zeros((NCORES, W_WIN), np.int64)
    hi_cnt = np.zeros((NCORES, W_WIN), np.int64)
    np.add.at(lo_cnt, (core, win), 1 - is_hi)
    np.add.at(hi_cnt, (core, win), is_hi)

    lo_T = np.maximum(1, -(-lo_cnt.max(axis=0) // P))
    hi_T = -(-hi_cnt.max(axis=0) // P)
    win_T = lo_T + hi_T
    T_total = int(win_T.sum())

    lo_base = np.zeros(W_WIN, np.int64)
    hi_base = np.zeros(W_WIN, np.int64)
    runs = []  # per window: [(tile_offset, n_tiles, is_hi), ...]
    t = 0
    for w in range(W_WIN):
        lo_base[w] = t * P
        wruns = [(t, int(lo_T[w]), 0)]
        t += int(lo_T[w])
        hi_base[w] = t * P
        if hi_T[w] > 0:
            wruns.append((t, int(hi_T[w]), 1))
        t += int(hi_T[w])
        runs.append(wruns)
    assert t == T_total

    idx_all = np.zeros((NCORES, T_total * P), np.int16)
    oh_val = np.zeros((NCORES, T_total * P), np.float32)
    oh_d8 = np.zeros((NCORES, T_total * P), np.int64)

    order = np.lexsort((is_hi, win, core))
    e_core, e_win, e_hi = core[order], win[order], is_hi[order]
    e_srcg, e_d8, e_val = srcg[order], d8[order], edge_val[order]

    grp = (e_core * W_WIN + e_win) * 2 + e_hi
    cnt = np.zeros(NCORES * W_WIN * 2 + 1, np.int64)
    np.add.at(cnt, grp + 1, 1)
    starts = np.cumsum(cnt)[:-1]
    within = np.arange(len(order)) - starts[grp]

    slot = np.where(e_hi == 0, lo_base[e_win], hi_base[e_win]) + within
    idx_all[e_core, slot] = (e_srcg - e_hi * HALFROWS).astype(np.int16)
    oh_val[e_core, slot] = e_val
    oh_d8[e_core, slot] = e_d8

    return dict(idx_all=idx_all, oh_val=oh_val, oh_d8=oh_d8, runs=runs,
                win_T=win_T, T_total=T_total, perms=perms)


def build_in_maps(inputs, pp):
    raw_x = np.ascontiguousarray(inputs["raw_x"], dtype=np.float32)
    T = pp["T_total"]

    def wblocks(w, fout_pad):
        wp = np.zeros((w.shape[0], fout_pad), np.float32)
        wp[:, :w.shape[1]] = w
        kh = w.shape[0] // P
        return wp.reshape(kh, P, fout_pad)

    W012 = np.stack([
        np.stack([np.stack([inputs[k][a * P:(a + 1) * P, b * P:(b + 1) * P]
                            for b in range(2)]) for a in range(2)])
        for k in ("W_gc0", "W_gc1", "W_gc2")]).astype(np.float32)
    W3 = wblocks(np.asarray(inputs["W_gc3"], np.float32), 128)
    Wr0 = np.stack([np.stack([inputs["W_res0"][a * P:(a + 1) * P, b * P:(b + 1) * P]
                              for b in range(2)]) for a in range(2)]).astype(np.float32)
    WrL = wblocks(np.asarray(inputs["W_res_last"], np.float32), 64)
    ident = np.eye(P, dtype=np.float32)

    in_maps = []
    for c in range(NCORES):
        xT0 = np.zeros((2, P, NPAD), np.float32)
        xloc = raw_x[c * NLOC:(c + 1) * NLOC]          # [NLOC, 256]
        xT0[:, :, pp["perms"][c]] = xloc.T.reshape(2, P, NLOC)

        idx16 = np.zeros((16, T * 8), np.int16)
        i = np.arange(T * P)
        idx16[i % 16, i // 16] = pp["idx_all"][c]
        gidx = np.tile(idx16, (8, 1))                  # [128, T*8]

        import ml_dtypes
        oh = np.zeros((T * P, P), np.float32)
        oh[np.arange(T * P), pp["oh_d8"][c]] = pp["oh_val"][c]
        oh = np.ascontiguousarray(
            oh.reshape(T, P, P).transpose(1, 0, 2)).astype(ml_dtypes.bfloat16)

        in_maps.append(dict(xT0=xT0, gidx=gidx, oh=oh, W012=W012, W3=W3,
                            Wr0=Wr0, WrL=WrL, ident=ident))
    return in_maps


# ------------------------------------------------------------- device program


def build_program(pp):
    T = pp["T_total"]
    win_T = pp["win_T"]
    runs = pp["runs"]
    Tmax = int(win_T.max())
    NG = [(g * 512, min(512, NPAD - g * 512)) for g in range(-(-NPAD // 512))]

    nc = bacc.Bacc("TRN2", target_bir_lowering=False, debug=False,
                   num_devices=NCORES)

    xT0_d = nc.dram_tensor("xT0", [2, P, NPAD], F32, kind="ExternalInput")
    gidx_d = nc.dram_tensor("gidx", [P, T * 8], I16, kind="ExternalInput")
    oh_d = nc.dram_tensor("oh", [P, T, P], BF16, kind="ExternalInput")
    W012_d = nc.dram_tensor("W012", [3, 2, 2, P, P], F32, kind="ExternalInput")
    W3_d = nc.dram_tensor("W3", [2, P, 128], F32, kind="ExternalInput")
    Wr0_d = nc.dram_tensor("Wr0", [2, 2, P, P], F32, kind="ExternalInput")
    WrL_d = nc.dram_tensor("WrL", [2, P, 64], F32, kind="ExternalInput")
    ident_d = nc.dram_tensor("ident", [P, P], F32, kind="ExternalInput")
    out_d = nc.dram_tensor("out", [NPAD, C_OUT], F32, kind="ExternalOutput")

    res0_d = nc.dram_tensor("res0_d", [NPAD, HID], F32)
    rlast_d = nc.dram_tensor("rlast_d", [NPAD, 64], F32)
    ag_in = [nc.dram_tensor(f"ag_in{i}", [NPAD, HID], BF16) for i in range(2)]
    hfull = [nc.dram_tensor(f"hfull{i}", [NCORES * NPAD, HID], BF16,
                            addr_space="Shared") for i in range(2)]
    ag3_in = nc.dram_tensor("ag3_in", [NPAD, 128], BF16)
    h3full = nc.dram_tensor("h3full", [NCORES * NPAD, 128], BF16,
                            addr_space="Shared")

    with tile.TileContext(nc) as tc:
        with (
            tc.tile_pool(name="pers", bufs=1) as pers,
            tc.tile_pool(name="wbufp", bufs=3) as wbufp,
            tc.tile_pool(name="ohp", bufs=3) as ohp,
            tc.tile_pool(name="hTs", bufs=3) as hTsp,
            tc.tile_pool(name="hns", bufs=4) as hnsp,
            tc.tile_pool(name="zs", bufs=4) as zsp,
            tc.tile_pool(name="r0s", bufs=4) as r0sp,
            tc.tile_pool(name="sm", bufs=6) as smp,
            tc.tile_pool(name="ps_s", bufs=3, space="PSUM") as ps_s,
            tc.tile_pool(name="ps_d", bufs=2, space="PSUM") as ps_d,
            tc.tile_pool(name="ps_t", bufs=2, space="PSUM") as ps_t,
        ):
            # ---------------- persistent loads
            xT = pers.tile([P, 2, NPAD], F32, tag="xT")
            nc.sync.dma_start(out=xT[:, 0, :], in_=xT0_d.ap()[0])
            nc.sync.dma_start(out=xT[:, 1, :], in_=xT0_d.ap()[1])
            idxs = pers.tile([P, T * 8], I16, tag="gidx")
            nc.sync.dma_start(out=idxs[:], in_=gidx_d.ap())
            w012 = pers.tile([P, 12, P], F32, tag="w012")
            nc.sync.dma_start(
                out=w012[:],
                in_=W012_d.ap().rearrange("a b c p m -> p (a b c) m"))
            w3 = pers.tile([P, 2, 128], F32, tag="w3")
            nc.sync.dma_start(out=w3[:], in_=W3_d.ap().rearrange("a p m -> p a m"))
            wr0 = pers.tile([P, 4, P], F32, tag="wr0")
            nc.sync.dma_start(out=wr0[:],
                              in_=Wr0_d.ap().rearrange("a b p m -> p (a b) m"))
            wrl = pers.tile([P, 2, 64], F32, tag="wrl")
            nc.sync.dma_start(out=wrl[:], in_=WrL_d.ap().rearrange("a p m -> p a m"))
            ident = pers.tile([P, P], F32, tag="ident")
            nc.sync.dma_start(out=ident[:], in_=ident_d.ap())

            def w012_ap(li, kh, fh):
                return w012[:, li * 4 + kh * 2 + fh, :]

            # ------------- dense helper: h[:, fh*128:...] = x @ W  (+ hooks)
            def dense256(w_ap_fn, dest_dram, rl_hook=None, hn_dt=BF16,
                         hn_tag="hn"):
                """w_ap_fn(kh, fh) -> lhsT [128,128]. Writes node-major
                [NPAD, 256] to dest_dram. rl_hook(fh, g0, ng, hT_stage)."""
                for g0, ng in NG:
                    stages = []
                    for fh in range(2):
                        psd = ps_d.tile([P, 512], F32, space="PSUM", tag="dps")
                        for kh in range(2):
                            nc.tensor.matmul(
                                psd[:, :ng], lhsT=w_ap_fn(kh, fh),
                                rhs=xT[:, kh, g0:g0 + ng],
                                start=(kh == 0), stop=(kh == 1))
                        hTst = hTsp.tile([P, 512], F32, tag="hT")
                        nc.scalar.copy(out=hTst[:, :ng], in_=psd[:, :ng])
                        stages.append(hTst)
                        if rl_hook is not None:
                            rl_hook(fh, g0, ng, hTst)
                    for s in range(ng // P):
                        hn = hnsp.tile([P, HID], hn_dt, tag=hn_tag)
                        for fh in range(2):
                            pst = ps_t.tile([P, P], F32, space="PSUM", tag="tp")
                            nc.tensor.transpose(
                                out=pst[:],
                                in_=stages[fh][:, s * P:(s + 1) * P],
                                identity=ident[:])
                            nc.vector.tensor_copy(
                                out=hn[:, fh * P:(fh + 1) * P], in_=pst[:])
                        nc.sync.dma_start(
                            out=dest_dram.ap()[g0 + s * P:g0 + (s + 1) * P, :],
                            in_=hn[:])

            # ---------------- init: res0 (+ rlast) from raw_x
            rl_ps = {}

            def rl_hook(fh, g0, ng, hTst):
                if fh == 0:
                    rl_ps["t"] = ps_d.tile([P, 512], F32, space="PSUM",
                                           tag="dps", name="psr")
                psr = rl_ps["t"]
                nc.tensor.matmul(psr[:64, :ng], lhsT=wrl[:, fh, :],
                                 rhs=hTst[:, :ng],
                                 start=(fh == 0), stop=(fh == 1))
                if fh == 1:
                    rlT = hTsp.tile([P, 512], F32, tag="hT")
                    nc.scalar.copy(out=rlT[:64, :ng], in_=psr[:64, :ng])
                    for s in range(ng // P):
                        pst = ps_t.tile([P, P], F32, space="PSUM", tag="tp")
                        nc.tensor.transpose(out=pst[:, :64],
                                            in_=rlT[:64, s * P:(s + 1) * P],
                                            identity=ident[:64, :64])
                        rln = hnsp.tile([P, 64], F32, tag="rln")
                        nc.vector.tensor_copy(out=rln[:], in_=pst[:, :64])
                        nc.sync.dma_start(
                            out=rlast_d.ap()[g0 + s * P:g0 + (s + 1) * P, :],
                            in_=rln[:])

            dense256(lambda kh, fh: wr0[:, kh * 2 + fh, :], res0_d,
                     rl_hook=rl_hook, hn_dt=F32, hn_tag="hnf")

            # ---------------- spmm helper
            def spmm(hfull_t, elem, psw, evict_fn):
                pos = 0
                for w in range(W_WIN):
                    nT = int(win_T[w])
                    ohb = ohp.tile([P, Tmax, P], BF16, tag="ohb")
                    nc.sync.dma_start(out=ohb[:, :nT, :],
                                      in_=oh_d.ap()[:, pos:pos + nT, :])
                    wb = wbufp.tile([P, Tmax, elem], BF16, tag="wb")
                    if SKIP_GATHER:
                        nc.vector.memset(wb[:, 0, :], 0.0)
                    gq = [0]
                    for (t0, nt, hi) in runs[w]:
                        src = (hfull_t.ap()[HALFROWS:, :] if hi
                               else hfull_t.ap()[:HALFROWS, :])
                        for c0 in range(0, nt, MAX_GATHER_TILES):
                            cn = min(MAX_GATHER_TILES, nt - c0)
                            toff = t0 - pos + c0
                            tg = t0 + c0
                            if SKIP_GATHER:
                                continue
                            nc.gpsimd.dma_gather(
                                wb[:, toff:toff + cn, :], src,
                                idxs[:, tg * 8:(tg + cn) * 8],
                                cn * P, cn * P, elem,
                                single_packet=GATHER_SINGLE_PACKET,
                                queue_num=gq[0])
                            gq[0] = (gq[0] + 1) % GATHER_QUEUES
                    pss = ps_s.tile([P, HID], F32, space="PSUM", tag="sps")
                    if SKIP_SPMM_MM:
                        nc.tensor.matmul(pss[:, :psw], lhsT=ohb[:, 0, :],
                                         rhs=wb[:, 0, :], start=True, stop=True)
                    else:
                        for t in range(nT):
                            nc.tensor.matmul(pss[:, :psw],
                                             lhsT=ohb[:, t, :],
                                             rhs=wb[:, t, :],
                                             start=(t == 0), stop=(t == nT - 1))
                    evict_fn(w, pss)
                    pos += nT

            # ---------------- GCN layers 0..2
            for li in range(3):
                dense256(lambda kh, fh, li=li: w012_ap(li, kh, fh), ag_in[li % 2])
                nc.gpsimd.collective_compute(
                    "AllGather", mybir.AluOpType.bypass,
                    replica_groups=[list(range(NCORES))],
                    ins=[ag_in[li % 2].ap()], outs=[hfull[li % 2].ap()])

                def evict_gc(w, pss):
                    r0 = r0sp.tile([P, HID], F32, tag="r0")
                    nc.sync.dma_start(out=r0[:],
                                      in_=res0_d.ap()[w * P:(w + 1) * P, :])
                    z = zsp.tile([P, HID], F32, tag="z")
                    nc.vector.tensor_add(out=z[:], in0=pss[:], in1=r0[:])
                    for fh in range(2):
                        pst = ps_t.tile([P, P], F32, space="PSUM", tag="tp")
                        nc.tensor.transpose(out=pst[:],
                                            in_=z[:, fh * P:(fh + 1) * P],
                                            identity=ident[:])
                        nc.scalar.activation(
                            out=xT[:, fh, w * P:(w + 1) * P], in_=pst[:],
                            func=mybir.ActivationFunctionType.Relu)

                spmm(hfull[li % 2], HID, HID, evict_gc)

            # ---------------- layer 3 dense (fout=64 incl. padding)
            for g0, ng in NG:
                psd = ps_d.tile([P, 512], F32, space="PSUM", tag="dps")
                for kh in range(2):
                    nc.tensor.matmul(psd[:, :ng], lhsT=w3[:, kh, :],
                                     rhs=xT[:, kh, g0:g0 + ng],
                                     start=(kh == 0), stop=(kh == 1))
                h3T = hTsp.tile([P, 512], F32, tag="hT")
                nc.scalar.copy(out=h3T[:, :ng], in_=psd[:, :ng])
                for s in range(ng // P):
                    pst = ps_t.tile([P, P], F32, space="PSUM", tag="tp")
                    nc.tensor.transpose(out=pst[:],
                                        in_=h3T[:, s * P:(s + 1) * P],
                                        identity=ident[:])
                    h3n = hnsp.tile([P, 128], BF16, tag="h3n")
                    nc.vector.tensor_copy(out=h3n[:], in_=pst[:])
                    nc.sync.dma_start(
                        out=ag3_in.ap()[g0 + s * P:g0 + (s + 1) * P, :],
                        in_=h3n[:])
            nc.gpsimd.collective_compute(
                "AllGather", mybir.AluOpType.bypass,
                replica_groups=[list(range(NCORES))],
                ins=[ag3_in.ap()], outs=[h3full.ap()])

            # ---------------- layer 3 spmm + rlast + log_softmax
            def evict_out(w, pss):
                rl = r0sp.tile([P, 64], F32, tag="rl")
                nc.sync.dma_start(out=rl[:],
                                  in_=rlast_d.ap()[w * P:(w + 1) * P, :])
                y = zsp.tile([P, 64], F32, tag="y")
                nc.vector.tensor_add(out=y[:], in0=pss[:, :64], in1=rl[:])
                m = smp.tile([P, 1], F32, tag="m")
                nc.vector.tensor_reduce(out=m[:], in_=y[:, :C_OUT],
                                        axis=mybir.AxisListType.X,
                                        op=mybir.AluOpType.max)
                tt = smp.tile([P, C_OUT], F32, tag="tt")
                nc.vector.tensor_scalar(out=tt[:], in0=y[:, :C_OUT],
                                        scalar1=m[:], scalar2=None,
                                        op0=mybir.AluOpType.subtract)
                e = smp.tile([P, C_OUT], F32, tag="e")
                ssum = smp.tile([P, 1], F32, tag="ss")
                nc.scalar.activation(out=e[:], in_=tt[:],
                                     func=mybir.ActivationFunctionType.Exp,
                                     accum_out=ssum[:])
                lg = smp.tile([P, 1], F32, tag="lg")
                nc.scalar.activation(out=lg[:], in_=ssum[:],
                                     func=mybir.ActivationFunctionType.Ln)
                o = smp.tile([P, C_OUT], F32, tag="o")
                nc.vector.tensor_scalar(out=o[:], in0=tt[:], scalar1=lg[:],
                                        scalar2=None,
                                        op0=mybir.AluOpType.subtract)
                nc.sync.dma_start(out=out_d.ap()[w * P:(w + 1) * P, :],
                                  in_=o[:])

            spmm(h3full, 128, 128, evict_out)

    nc.compile()
    return nc


# ------------------------------------------------------------ timed benchmark


def bench(nc, in_maps, iters=8):
    """Times repeated on-device executions with device-resident inputs
    (replicates bass2jax.run_bass_via_pjrt's multi-core path). Returns
    (best_seconds, per_iter_seconds)."""
    import time

    import jax
    from jax.experimental.shard_map import shard_map
    from jax.sharding import Mesh, NamedSharding, PartitionSpec

    from concourse import bass2jax, mybir as mb

    bass2jax.install_neuronx_cc_hook()

    partition_name = (nc.partition_id_tensor.name
                      if nc.partition_id_tensor else None)
    in_names, out_names, out_avals, zero_outs = [], [], [], []
    for alloc in nc.m.functions[0].allocations:
        if not isinstance(alloc, mb.MemoryLocationSet):
            continue
        name = alloc.memorylocations[0].name
        if alloc.kind == "ExternalInput":
            if name != partition_name:
                in_names.append(name)
        elif alloc.kind == "ExternalOutput":
            out_names.append(name)
            shape = tuple(alloc.tensor_shape)
            dtype = mb.dt.np(alloc.dtype)
            out_avals.append(jax.core.ShapedArray(shape, dtype))
            zero_outs.append(np.zeros(shape, dtype))
    n_params = len(in_names)
    n_outs = len(out_avals)
    all_names = in_names + out_names
    if partition_name is not None:
        all_names = all_names + [partition_name]

    def _body(*args):
        operands = list(args)
        if partition_name is not None:
            operands.append(bass2jax.partition_id_tensor())
        outs = bass2jax._bass_exec_p.bind(
            *operands, out_avals=tuple(out_avals), in_names=tuple(all_names),
            out_names=tuple(out_names), lowering_input_output_aliases=(),
            sim_require_finite=True, sim_require_nnan=True, nc=nc)
        return tuple(outs)

    devices = jax.devices()[:NCORES]
    mesh = Mesh(np.asarray(devices), ("core",))
    in_specs = (PartitionSpec("core"),) * (n_params + n_outs)
    out_specs = (PartitionSpec("core"),) * n_outs
    donate = tuple(range(n_params, n_params + n_outs))
    del donate
    sharded = jax.jit(shard_map(_body, mesh=mesh, in_specs=in_specs,
                                out_specs=out_specs, check_rep=False),
                      keep_unused=True)

    sh = NamedSharding(mesh, PartitionSpec("core"))
    dev_in = [
        jax.device_put(
            np.concatenate([np.asarray(in_maps[c][n]) for c in range(NCORES)],
                           axis=0), sh)
        for n in in_names]
    zglobal = [np.zeros((NCORES * z.shape[0], *z.shape[1:]), z.dtype)
               for z in zero_outs]

    dz = [jax.device_put(z, sh) for z in zglobal]
    for d in dz:
        d.block_until_ready()

    def run_batch(n):
        """Queue n executions without intermediate sync; return elapsed."""
        t0 = time.perf_counter()
        outs = None
        for _ in range(n):
            outs = sharded(*dev_in, *dz)
        for o in outs:
            o.block_until_ready()
        return time.perf_counter() - t0

    run_batch(1)  # warmup
    n_hi = 41
    lo = min(run_batch(1) for _ in range(iters))
    hi = min(run_batch(n_hi) for _ in range(max(2, iters // 2)))
    per_exec = (hi - lo) / (n_hi - 1)
    return per_exec, (lo, hi)


# ---------------------------------------------------------------- entry point

_CACHE = {}


def _run(inputs, trace=False, trace_kwargs=None):
    pp = preprocess(np.asarray(inputs["edge_row"]),
                    np.asarray(inputs["edge_col"]),
                    np.asarray(inputs["edge_val"], dtype=np.float32))
    in_maps = build_in_maps(inputs, pp)
    key = (pp["T_total"], tuple(tuple(ws) for ws in pp["runs"]))
    if key not in _CACHE:
        _CACHE.clear()
        _CACHE[key] = build_program(pp)
    nc = _CACHE[key]
    res = run_bass_kernel_spmd(nc, in_maps, list(range(NCORES)), trace=trace,
                               **(trace_kwargs or {}))
    outs = [res.results[c]["out"][pp["perms"][c]] for c in range(NCORES)]
    full = np.concatenate(outs, axis=0).astype(np.float32)
    return full, res


def kernel(**inputs):
    out, _ = _run(inputs)
    return out


# revision 23
# speedup vs baseline: 11.5220x; 1.0434x over previous
"""DeepGCN ResNet (4-layer GCN w/ residuals + log_softmax) on 8 TRN2 NeuronCores.

Sharding: nodes (rows) split 8 ways; edges partitioned by destination row.
Per layer: local dense x@W -> AllGather h -> dma_gather of source rows ->
one-hot segment-matmul on TensorE (PSUM accumulate per 128-dest window) ->
+res0, relu. Final layer + res0@W_res_last + log_softmax.

Host-side preprocessing only does layout work: edge sorting by (dest window,
src half), padding to a core-uniform tile structure, one-hot tile
construction, and int16 gather-index packing (sources split in two halves so
row indices fit int16).
"""

import numpy as np

import concourse.bacc as bacc
import concourse.mybir as mybir
import concourse.tile as tile
from concourse.bass_utils import run_bass_kernel_spmd

P = 128

# Problem geometry (hardcoded per the task contract).
N_NODES = 50000
N_EDGES = 800000
F_IN = 256
HID = 256
C_OUT = 40
NCORES = 8

NLOC = N_NODES // NCORES            # 6250
NPAD = ((NLOC + P - 1) // P) * P    # 6272
W_WIN = NPAD // P                   # 49 windows of 128 dest rows
HALFROWS = (NCORES // 2) * NPAD     # 25088 (< int16 max) split of h_full rows


def _set_geometry(n_nodes):
    """Debug hook: shrink the node count (keeps F/HID/C). Used only by the
    small-scale simulator test, never in grading."""
    global N_NODES, NLOC, NPAD, W_WIN, HALFROWS
    N_NODES = n_nodes
    NLOC = N_NODES // NCORES
    NPAD = ((NLOC + P - 1) // P) * P
    W_WIN = NPAD // P
    HALFROWS = (NCORES // 2) * NPAD

F32 = mybir.dt.float32
BF16 = mybir.dt.bfloat16
I16 = mybir.dt.int16
MAX_GATHER_TILES = 999
GATHER_SINGLE_PACKET = False
SKIP_GATHER = False      # timing bisect: omit dma_gather calls
SKIP_SPMM_MM = False     # timing bisect: omit segment matmuls
GATHER_QUEUES = 1        # rotate dma_gather queue_num over this many queues


# ----------------------------------------------------------------- host prep


def preprocess(edge_row, edge_col, edge_val):
    """Edge partitioning/sorting/padding with per-core dest->window
    rebalancing (greedy 2D bin-pack on lo/hi in-degree). Returns per-core
    data arrays plus a core-uniform tile structure."""
    edge_row = edge_row.astype(np.int64)
    edge_col = edge_col.astype(np.int64)

    # --- lo/hi in-degree per destination (lo = src owned by cores 0..3)
    src_core0 = edge_col // NLOC
    src_hi0 = src_core0 >= (NCORES // 2)
    lo_deg = np.zeros(N_NODES, np.int64)
    hi_deg = np.zeros(N_NODES, np.int64)
    np.add.at(lo_deg, edge_row, ~src_hi0)
    np.add.at(hi_deg, edge_row, src_hi0)

    # --- per-core greedy assignment of dests to windows (balance lo & hi)
    pos_of_node = np.zeros(N_NODES, np.int64)
    perms = []  # per core: padded_pos[d_local]
    for c in range(NCORES):
        ld = lo_deg[c * NLOC:(c + 1) * NLOC].astype(np.float64)
        hd = hi_deg[c * NLOC:(c + 1) * NLOC].astype(np.float64)
        order = np.argsort(-(ld + hd), kind="stable")
        # windows 0..N_BIG-1 get a 9-tile budget (1152); rest hard-capped at
        # 1024 so they stay 8 tiles. Every core's excess lands in the same
        # window indices, so the cross-core max stays tight.
        N_BIG = 5
        cap = np.full(W_WIN, 8.0 * P)
        cap[:N_BIG] = 9.0 * P
        loads_lo = np.zeros(W_WIN)
        loads_hi = np.zeros(W_WIN)
        counts = np.zeros(W_WIN, np.int64)
        wassign = np.zeros(NLOC, np.int64)
        for t, d in enumerate(order):
            ccap = min(P, t // W_WIN + 2)  # stay within 2 of even fill
            cost = np.maximum(loads_lo + ld[d], loads_hi + hd[d])
            infeas = ((counts >= ccap) | (loads_lo + ld[d] > cap)
                      | (loads_hi + hd[d] > cap))
            if infeas.all():
                infeas = counts >= ccap
            if infeas.all():
                infeas = counts >= P
            cost = np.where(infeas, 1e18, cost)
            w = int(np.argmin(cost))
            wassign[d] = w
            loads_lo[w] += ld[d]
            loads_hi[w] += hd[d]
            counts[w] += 1
        # slot within window
        slot_in_w = np.zeros(NLOC, np.int64)
        fill = np.zeros(W_WIN, np.int64)
        for d in range(NLOC):
            w = wassign[d]
            slot_in_w[d] = fill[w]
            fill[w] += 1
        p = wassign * P + slot_in_w
        perms.append(p)
        pos_of_node[c * NLOC:(c + 1) * NLOC] = c * NPAD + p

    core = edge_row // NLOC
    p_local = pos_of_node[edge_row] - core * NPAD
    win = p_local // P
    d8 = p_local % P
    srcg = pos_of_node[edge_col]
    is_hi = (srcg >= HALFROWS).astype(np.int64)

    lo_cnt = np.zeros((NCORES, W_WIN), np.int64)
    hi_cnt = np.zeros((NCORES, W_WIN), np.int64)
    np.add.at(lo_cnt, (core, win), 1 - is_hi)
    np.add.at(hi_cnt, (core, win), is_hi)

    lo_T = np.maximum(1, -(-lo_cnt.max(axis=0) // P))
    hi_T = -(-hi_cnt.max(axis=0) // P)
    win_T = lo_T + hi_T
    T_total = int(win_T.sum())

    # Paired-window stream: [wA_lo | wB_lo | wA_hi | wB_hi] so one gather
    # call covers both windows' lo (resp. hi) tiles -> fewer, bigger calls.
    lo_base = np.zeros(W_WIN, np.int64)
    hi_base = np.zeros(W_WIN, np.int64)
    pairs = [tuple(range(p, min(p + 2, W_WIN))) for p in range(0, W_WIN, 2)]
    gcalls = []   # per pair: [(tile_offset, n_tiles, is_hi), ...]
    spans = {}    # window -> [(tile_offset, n_tiles), ...] for matmuls
    t = 0
    for pr in pairs:
        t0 = t
        for w in pr:
            lo_base[w] = t * P
            spans[w] = [(t, int(lo_T[w]))]
            t += int(lo_T[w])
        calls = [(t0, t - t0, 0)]
        t1 = t
        for w in pr:
            hi_base[w] = t * P
            if hi_T[w] > 0:
                spans[w].append((t, int(hi_T[w])))
            t += int(hi_T[w])
        if t > t1:
            calls.append((t1, t - t1, 1))
        gcalls.append(calls)
    assert t == T_total
    runs = None  # superseded by pairs/gcalls/spans

    idx_all = np.zeros((NCORES, T_total * P), np.int16)
    oh_val = np.zeros((NCORES, T_total * P), np.float32)
    oh_d8 = np.zeros((NCORES, T_total * P), np.int64)

    order = np.lexsort((is_hi, win, core))
    e_core, e_win, e_hi = core[order], win[order], is_hi[order]
    e_srcg, e_d8, e_val = srcg[order], d8[order], edge_val[order]

    grp = (e_core * W_WIN + e_win) * 2 + e_hi
    cnt = np.zeros(NCORES * W_WIN * 2 + 1, np.int64)
    np.add.at(cnt, grp + 1, 1)
    starts = np.cumsum(cnt)[:-1]
    within = np.arange(len(order)) - starts[grp]

    slot = np.where(e_hi == 0, lo_base[e_win], hi_base[e_win]) + within
    idx_all[e_core, slot] = (e_srcg - e_hi * HALFROWS).astype(np.int16)
    oh_val[e_core, slot] = e_val
    oh_d8[e_core, slot] = e_d8

    return dict(idx_all=idx_all, oh_val=oh_val, oh_d8=oh_d8, pairs=pairs,
                gcalls=gcalls, spans=spans, win_T=win_T, T_total=T_total,
                perms=perms)


def build_in_maps(inputs, pp):
    raw_x = np.ascontiguousarray(inputs["raw_x"], dtype=np.float32)
    T = pp["T_total"]

    def wblocks(w, fout_pad):
        wp = np.zeros((w.shape[0], fout_pad), np.float32)
        wp[:, :w.shape[1]] = w
        kh = w.shape[0] // P
        return wp.reshape(kh, P, fout_pad)

    W012 = np.stack([
        np.stack([np.stack([inputs[k][a * P:(a + 1) * P, b * P:(b + 1) * P]
                            for b in range(2)]) for a in range(2)])
        for k in ("W_gc0", "W_gc1", "W_gc2")]).astype(np.float32)
    W3 = wblocks(np.asarray(inputs["W_gc3"], np.float32), 128)
    Wr0 = np.stack([np.stack([inputs["W_res0"][a * P:(a + 1) * P, b * P:(b + 1) * P]
                              for b in range(2)]) for a in range(2)]).astype(np.float32)
    WrL = wblocks(np.asarray(inputs["W_res_last"], np.float32), 64)
    ident = np.eye(P, dtype=np.float32)

    in_maps = []
    for c in range(NCORES):
        xT0 = np.zeros((2, P, NPAD), np.float32)
        xloc = raw_x[c * NLOC:(c + 1) * NLOC]          # [NLOC, 256]
        xT0[:, :, pp["perms"][c]] = xloc.T.reshape(2, P, NLOC)

        idx16 = np.zeros((16, T * 8), np.int16)
        i = np.arange(T * P)
        idx16[i % 16, i // 16] = pp["idx_all"][c]
        gidx = np.tile(idx16, (8, 1))                  # [128, T*8]

        import ml_dtypes
        oh = np.zeros((T * P, P), np.float32)
        oh[np.arange(T * P), pp["oh_d8"][c]] = pp["oh_val"][c]
        oh = np.ascontiguousarray(
            oh.reshape(T, P, P).transpose(1, 0, 2)).astype(ml_dtypes.bfloat16)

        in_maps.append(dict(xT0=xT0, gidx=gidx, oh=oh, W012=W012, W3=W3,
                            Wr0=Wr0, WrL=WrL, ident=ident))
    return in_maps


# ------------------------------------------------------------- device program


def build_program(pp):
    T = pp["T_total"]
    win_T = pp["win_T"]
    pairs, gcalls, spans = pp["pairs"], pp["gcalls"], pp["spans"]
    Tmax = max(sum(int(win_T[w]) for w in pr) for pr in pairs)
    NG = [(g * 512, min(512, NPAD - g * 512)) for g in range(-(-NPAD // 512))]

    nc = bacc.Bacc("TRN2", target_bir_lowering=False, debug=False,
                   num_devices=NCORES)

    xT0_d = nc.dram_tensor("xT0", [2, P, NPAD], F32, kind="ExternalInput")
    gidx_d = nc.dram_tensor("gidx", [P, T * 8], I16, kind="ExternalInput")
    oh_d = nc.dram_tensor("oh", [P, T, P], BF16, kind="ExternalInput")
    W012_d = nc.dram_tensor("W012", [3, 2, 2, P, P], F32, kind="ExternalInput")
    W3_d = nc.dram_tensor("W3", [2, P, 128], F32, kind="ExternalInput")
    Wr0_d = nc.dram_tensor("Wr0", [2, 2, P, P], F32, kind="ExternalInput")
    WrL_d = nc.dram_tensor("WrL", [2, P, 64], F32, kind="ExternalInput")
    ident_d = nc.dram_tensor("ident", [P, P], F32, kind="ExternalInput")
    out_d = nc.dram_tensor("out", [NPAD, C_OUT], F32, kind="ExternalOutput")

    res0_d = nc.dram_tensor("res0_d", [NPAD, HID], F32)
    rlast_d = nc.dram_tensor("rlast_d", [NPAD, 64], F32)
    ag_in = [nc.dram_tensor(f"ag_in{i}", [NPAD, HID], BF16) for i in range(2)]
    hfull = [nc.dram_tensor(f"hfull{i}", [NCORES * NPAD, HID], BF16,
                            addr_space="Shared") for i in range(2)]
    ag3_in = nc.dram_tensor("ag3_in", [NPAD, 128], BF16)
    h3full = nc.dram_tensor("h3full", [NCORES * NPAD, 128], BF16,
                            addr_space="Shared")

    with tile.TileContext(nc) as tc:
        with (
            tc.tile_pool(name="pers", bufs=1) as pers,
            tc.tile_pool(name="wbufp", bufs=2) as wbufp,
            tc.tile_pool(name="ohp", bufs=2) as ohp,
            tc.tile_pool(name="hTs", bufs=3) as hTsp,
            tc.tile_pool(name="hns", bufs=4) as hnsp,
            tc.tile_pool(name="zs", bufs=4) as zsp,
            tc.tile_pool(name="r0s", bufs=4) as r0sp,
            tc.tile_pool(name="sm", bufs=6) as smp,
            tc.tile_pool(name="ps_s", bufs=4, space="PSUM") as ps_s,
            tc.tile_pool(name="ps_d", bufs=2, space="PSUM") as ps_d,
            tc.tile_pool(name="ps_t", bufs=2, space="PSUM") as ps_t,
        ):
            # ---------------- persistent loads
            xT = pers.tile([P, 2, NPAD], F32, tag="xT")
            nc.sync.dma_start(out=xT[:, 0, :], in_=xT0_d.ap()[0])
            nc.sync.dma_start(out=xT[:, 1, :], in_=xT0_d.ap()[1])
            idxs = pers.tile([P, T * 8], I16, tag="gidx")
            nc.sync.dma_start(out=idxs[:], in_=gidx_d.ap())
            w012 = pers.tile([P, 12, P], F32, tag="w012")
            nc.sync.dma_start(
                out=w012[:],
                in_=W012_d.ap().rearrange("a b c p m -> p (a b c) m"))
            w3 = pers.tile([P, 2, 128], F32, tag="w3")
            nc.sync.dma_start(out=w3[:], in_=W3_d.ap().rearrange("a p m -> p a m"))
            wr0 = pers.tile([P, 4, P], F32, tag="wr0")
            nc.sync.dma_start(out=wr0[:],
                              in_=Wr0_d.ap().rearrange("a b p m -> p (a b) m"))
            wrl = pers.tile([P, 2, 64], F32, tag="wrl")
            nc.sync.dma_start(out=wrl[:], in_=WrL_d.ap().rearrange("a p m -> p a m"))
            ident = pers.tile([P, P], F32, tag="ident")
            nc.sync.dma_start(out=ident[:], in_=ident_d.ap())

            def w012_ap(li, kh, fh):
                return w012[:, li * 4 + kh * 2 + fh, :]

            # ------------- dense helper: h[:, fh*128:...] = x @ W  (+ hooks)
            def dense256(w_ap_fn, dest_dram, rl_hook=None, hn_dt=BF16,
                         hn_tag="hn"):
                """w_ap_fn(kh, fh) -> lhsT [128,128]. Writes node-major
                [NPAD, 256] to dest_dram. rl_hook(fh, g0, ng, hT_stage)."""
                for g0, ng in NG:
                    stages = []
                    for fh in range(2):
                        psd = ps_d.tile([P, 512], F32, space="PSUM", tag="dps")
                        for kh in range(2):
                            nc.tensor.matmul(
                                psd[:, :ng], lhsT=w_ap_fn(kh, fh),
                                rhs=xT[:, kh, g0:g0 + ng],
                                start=(kh == 0), stop=(kh == 1))
                        hTst = hTsp.tile([P, 512], F32, tag="hT")
                        nc.scalar.copy(out=hTst[:, :ng], in_=psd[:, :ng])
                        stages.append(hTst)
                        if rl_hook is not None:
                            rl_hook(fh, g0, ng, hTst)
                    for s in range(ng // P):
                        hn = hnsp.tile([P, HID], hn_dt, tag=hn_tag)
                        for fh in range(2):
                            pst = ps_t.tile([P, P], F32, space="PSUM", tag="tp")
                            nc.tensor.transpose(
                                out=pst[:],
                                in_=stages[fh][:, s * P:(s + 1) * P],
                                identity=ident[:])
                            nc.vector.tensor_copy(
                                out=hn[:, fh * P:(fh + 1) * P], in_=pst[:])
                        nc.sync.dma_start(
                            out=dest_dram.ap()[g0 + s * P:g0 + (s + 1) * P, :],
                            in_=hn[:])

            # ---------------- init: res0 (+ rlast) from raw_x
            rl_ps = {}

            def rl_hook(fh, g0, ng, hTst):
                if fh == 0:
                    rl_ps["t"] = ps_d.tile([P, 512], F32, space="PSUM",
                                           tag="dps", name="psr")
                psr = rl_ps["t"]
                nc.tensor.matmul(psr[:64, :ng], lhsT=wrl[:, fh, :],
                                 rhs=hTst[:, :ng],
                                 start=(fh == 0), stop=(fh == 1))
                if fh == 1:
                    rlT = hTsp.tile([P, 512], F32, tag="hT")
                    nc.scalar.copy(out=rlT[:64, :ng], in_=psr[:64, :ng])
                    for s in range(ng // P):
                        pst = ps_t.tile([P, P], F32, space="PSUM", tag="tp")
                        nc.tensor.transpose(out=pst[:, :64],
                                            in_=rlT[:64, s * P:(s + 1) * P],
                                            identity=ident[:64, :64])
                        rln = hnsp.tile([P, 64], F32, tag="rln")
                        nc.vector.tensor_copy(out=rln[:], in_=pst[:, :64])
                        nc.sync.dma_start(
                            out=rlast_d.ap()[g0 + s * P:g0 + (s + 1) * P, :],
                            in_=rln[:])

            dense256(lambda kh, fh: wr0[:, kh * 2 + fh, :], res0_d,
                     rl_hook=rl_hook, hn_dt=F32, hn_tag="hnf")

            # ---------------- spmm helper (paired windows)
            owner = {}
            for w, sp in spans.items():
                for (t0, nt) in sp:
                    for t in range(t0, t0 + nt):
                        owner[t] = w

            def spmm(hfull_t, elem, psw, evict_fn):
                for ip, pr in enumerate(pairs):
                    p0 = spans[pr[0]][0][0]
                    nTp = sum(int(win_T[w]) for w in pr)
                    ohb = ohp.tile([P, Tmax, P], BF16, tag="ohb")
                    nc.sync.dma_start(out=ohb[:, :nTp, :],
                                      in_=oh_d.ap()[:, p0:p0 + nTp, :])
                    wb = wbufp.tile([P, Tmax, elem], BF16, tag="wb")
                    if SKIP_GATHER:
                        nc.vector.memset(wb[:, 0, :], 0.0)
                    for (t0, nt, hi) in gcalls[ip]:
                        if SKIP_GATHER:
                            continue
                        src = (hfull_t.ap()[HALFROWS:, :] if hi
                               else hfull_t.ap()[:HALFROWS, :])
                        nc.gpsimd.dma_gather(
                            wb[:, t0 - p0:t0 - p0 + nt, :], src,
                            idxs[:, t0 * 8:(t0 + nt) * 8],
                            nt * P, nt * P, elem,
                            single_packet=GATHER_SINGLE_PACKET)
                    psl = {w: ps_s.tile([P, HID], F32, space="PSUM",
                                        tag="sps", name=f"pss_{ip}_{w}")
                           for w in pr}
                    cnt = {w: 0 for w in pr}
                    for t in range(p0, p0 + nTp):
                        w = owner[t]
                        nc.tensor.matmul(psl[w][:, :psw],
                                         lhsT=ohb[:, t - p0, :],
                                         rhs=wb[:, t - p0, :],
                                         start=(cnt[w] == 0),
                                         stop=(cnt[w] == int(win_T[w]) - 1))
                        cnt[w] += 1
                    for w in pr:
                        evict_fn(w, psl[w])

            # ---------------- GCN layers 0..2
            for li in range(3):
                dense256(lambda kh, fh, li=li: w012_ap(li, kh, fh), ag_in[li % 2])
                nc.gpsimd.collective_compute(
                    "AllGather", mybir.AluOpType.bypass,
                    replica_groups=[list(range(NCORES))],
                    ins=[ag_in[li % 2].ap()], outs=[hfull[li % 2].ap()])

                def evict_gc(w, pss):
                    r0 = r0sp.tile([P, HID], F32, tag="r0")
                    nc.sync.dma_start(out=r0[:],
                                      in_=res0_d.ap()[w * P:(w + 1) * P, :])
                    z = zsp.tile([P, HID], F32, tag="z")
                    nc.vector.tensor_add(out=z[:], in0=pss[:], in1=r0[:])
                    for fh in range(2):
                        pst = ps_t.tile([P, P], F32, space="PSUM", tag="tp")
                        nc.tensor.transpose(out=pst[:],
                                            in_=z[:, fh * P:(fh + 1) * P],
                                            identity=ident[:])
                        nc.scalar.activation(
                            out=xT[:, fh, w * P:(w + 1) * P], in_=pst[:],
                            func=mybir.ActivationFunctionType.Relu)

                spmm(hfull[li % 2], HID, HID, evict_gc)

            # ---------------- layer 3 dense (fout=64 incl. padding)
            for g0, ng in NG:
                psd = ps_d.tile([P, 512], F32, space="PSUM", tag="dps")
                for kh in range(2):
                    nc.tensor.matmul(psd[:, :ng], lhsT=w3[:, kh, :],
                                     rhs=xT[:, kh, g0:g0 + ng],
                                     start=(kh == 0), stop=(kh == 1))
                h3T = hTsp.tile([P, 512], F32, tag="hT")
                nc.scalar.copy(out=h3T[:, :ng], in_=psd[:, :ng])
                for s in range(ng // P):
                    pst = ps_t.tile([P, P], F32, space="PSUM", tag="tp")
                    nc.tensor.transpose(out=pst[:],
                                        in_=h3T[:, s * P:(s + 1) * P],
                                        identity=ident[:])
                    h3n = hnsp.tile([P, 128], BF16, tag="h3n")
                    nc.vector.tensor_copy(out=h3n[:], in_=pst[:])
                    nc.sync.dma_start(
                        out=ag3_in.ap()[g0 + s * P:g0 + (s + 1) * P, :],
                        in_=h3n[:])
            nc.gpsimd.collective_compute(
                "AllGather", mybir.AluOpType.bypass,
                replica_groups=[list(range(NCORES))],
                ins=[ag3_in.ap()], outs=[h3full.ap()])

            # ---------------- layer 3 spmm + rlast + log_softmax
            def evict_out(w, pss):
                rl = r0sp.tile([P, 64], F32, tag="rl")
                nc.sync.dma_start(out=rl[:],
                                  in_=rlast_d.ap()[w * P:(w + 1) * P, :])
                y = zsp.tile([P, 64], F32, tag="y")
                nc.vector.tensor_add(out=y[:], in0=pss[:, :64], in1=rl[:])
                m = smp.tile([P, 1], F32, tag="m")
                nc.vector.tensor_reduce(out=m[:], in_=y[:, :C_OUT],
                                        axis=mybir.AxisListType.X,
                                        op=mybir.AluOpType.max)
                tt = smp.tile([P, C_OUT], F32, tag="tt")
                nc.vector.tensor_scalar(out=tt[:], in0=y[:, :C_OUT],
                                        scalar1=m[:], scalar2=None,
                                        op0=mybir.AluOpType.subtract)
                e = smp.tile([P, C_OUT], F32, tag="e")
                ssum = smp.tile([P, 1], F32, tag="ss")
                nc.scalar.activation(out=e[:], in_=tt[:],
                                     func=mybir.ActivationFunctionType.Exp,
                                     accum_out=ssum[:])
                lg = smp.tile([P, 1], F32, tag="lg")
                nc.scalar.activation(out=lg[:], in_=ssum[:],
                                     func=mybir.ActivationFunctionType.Ln)
                o = smp.tile([P, C_OUT], F32, tag="o")
                nc.vector.tensor_scalar(out=o[:], in0=tt[:], scalar1=lg[:],
                                        scalar2=None,
                                        op0=mybir.AluOpType.subtract)
                nc.sync.dma_start(out=out_d.ap()[w * P:(w + 1) * P, :],
                                  in_=o[:])

            spmm(h3full, 128, 128, evict_out)

    nc.compile()
    return nc


# ------------------------------------------------------------ timed benchmark


def bench(nc, in_maps, iters=8):
    """Times repeated on-device executions with device-resident inputs
    (replicates bass2jax.run_bass_via_pjrt's multi-core path). Returns
    (best_seconds, per_iter_seconds)."""
    import time

    import jax
    from jax.experimental.shard_map import shard_map
    from jax.sharding import Mesh, NamedSharding, PartitionSpec

    from concourse import bass2jax, mybir as mb

    bass2jax.install_neuronx_cc_hook()

    partition_name = (nc.partition_id_tensor.name
                      if nc.partition_id_tensor else None)
    in_names, out_names, out_avals, zero_outs = [], [], [], []
    for alloc in nc.m.functions[0].allocations:
        if not isinstance(alloc, mb.MemoryLocationSet):
            continue
        name = alloc.memorylocations[0].name
        if alloc.kind == "ExternalInput":
            if name != partition_name:
                in_names.append(name)
        elif alloc.kind == "ExternalOutput":
            out_names.append(name)
            shape = tuple(alloc.tensor_shape)
            dtype = mb.dt.np(alloc.dtype)
            out_avals.append(jax.core.ShapedArray(shape, dtype))
            zero_outs.append(np.zeros(shape, dtype))
    n_params = len(in_names)
    n_outs = len(out_avals)
    all_names = in_names + out_names
    if partition_name is not None:
        all_names = all_names + [partition_name]

    def _body(*args):
        operands = list(args)
        if partition_name is not None:
            operands.append(bass2jax.partition_id_tensor())
        outs = bass2jax._bass_exec_p.bind(
            *operands, out_avals=tuple(out_avals), in_names=tuple(all_names),
            out_names=tuple(out_names), lowering_input_output_aliases=(),
            sim_require_finite=True, sim_require_nnan=True, nc=nc)
        return tuple(outs)

    devices = jax.devices()[:NCORES]
    mesh = Mesh(np.asarray(devices), ("core",))
    in_specs = (PartitionSpec("core"),) * (n_params + n_outs)
    out_specs = (PartitionSpec("core"),) * n_outs
    donate = tuple(range(n_params, n_params + n_outs))
    del donate
    sharded = jax.jit(shard_map(_body, mesh=mesh, in_specs=in_specs,
                                out_specs=out_specs, check_rep=False),
                      keep_unused=True)

    sh = NamedSharding(mesh, PartitionSpec("core"))
    dev_in = [
        jax.device_put(
            np.concatenate([np.asarray(in_maps[c][n]) for c in range(NCORES)],
                           axis=0), sh)
        for n in in_names]
    zglobal = [np.zeros((NCORES * z.shape[0], *z.shape[1:]), z.dtype)
               for z in zero_outs]

    dz = [jax.device_put(z, sh) for z in zglobal]
    for d in dz:
        d.block_until_ready()

    def run_batch(n):
        """Queue n executions without intermediate sync; return elapsed."""
        t0 = time.perf_counter()
        outs = None
        for _ in range(n):
            outs = sharded(*dev_in, *dz)
        for o in outs:
            o.block_until_ready()
        return time.perf_counter() - t0

    run_batch(1)  # warmup
    n_hi = 41
    lo = min(run_batch(1) for _ in range(iters))
    hi = min(run_batch(n_hi) for _ in range(max(2, iters // 2)))
    per_exec = (hi - lo) / (n_hi - 1)
    return per_exec, (lo, hi)


# ---------------------------------------------------------------- entry point

_CACHE = {}


def _run(inputs, trace=False, trace_kwargs=None):
    pp = preprocess(np.asarray(inputs["edge_row"]),
                    np.asarray(inputs["edge_col"]),
                    np.asarray(inputs["edge_val"], dtype=np.float32))
    in_maps = build_in_maps(inputs, pp)
    key = (pp["T_total"], tuple(tuple(c) for cs in pp["gcalls"] for c in cs))
    if key not in _CACHE:
        _CACHE.clear()
        _CACHE[key] = build_program(pp)
    nc = _CACHE[key]
    res = run_bass_kernel_spmd(nc, in_maps, list(range(NCORES)), trace=trace,
                               **(trace_kwargs or {}))
    outs = [res.results[c]["out"][pp["perms"][c]] for c in range(NCORES)]
    full = np.concatenate(outs, axis=0).astype(np.float32)
    return full, res


def kernel(**inputs):
    out, _ = _run(inputs)
    return out


# revision 24
# speedup vs baseline: 11.5881x; 1.0057x over previous
"""DeepGCN ResNet (4-layer GCN w/ residuals + log_softmax) on 8 TRN2 NeuronCores.

Sharding: nodes (rows) split 8 ways; edges partitioned by destination row.
Per layer: local dense x@W -> AllGather h -> dma_gather of source rows ->
one-hot segment-matmul on TensorE (PSUM accumulate per 128-dest window) ->
+res0, relu. Final layer + res0@W_res_last + log_softmax.

Host-side preprocessing only does layout work: edge sorting by (dest window,
src half), padding to a core-uniform tile structure, one-hot tile
construction, and int16 gather-index packing (sources split in two halves so
row indices fit int16).
"""

import numpy as np

import concourse.bacc as bacc
import concourse.mybir as mybir
import concourse.tile as tile
from concourse.bass_utils import run_bass_kernel_spmd

P = 128

# Problem geometry (hardcoded per the task contract).
N_NODES = 50000
N_EDGES = 800000
F_IN = 256
HID = 256
C_OUT = 40
NCORES = 8

NLOC = N_NODES // NCORES            # 6250
NPAD = ((NLOC + P - 1) // P) * P    # 6272
W_WIN = NPAD // P                   # 49 windows of 128 dest rows
HALFROWS = (NCORES // 2) * NPAD     # 25088 (< int16 max) split of h_full rows


def _set_geometry(n_nodes):
    """Debug hook: shrink the node count (keeps F/HID/C). Used only by the
    small-scale simulator test, never in grading."""
    global N_NODES, NLOC, NPAD, W_WIN, HALFROWS
    N_NODES = n_nodes
    NLOC = N_NODES // NCORES
    NPAD = ((NLOC + P - 1) // P) * P
    W_WIN = NPAD // P
    HALFROWS = (NCORES // 2) * NPAD

F32 = mybir.dt.float32
BF16 = mybir.dt.bfloat16
I16 = mybir.dt.int16
MAX_GATHER_TILES = 999
GATHER_SINGLE_PACKET = False
SKIP_GATHER = False      # timing bisect: omit dma_gather calls
SKIP_SPMM_MM = False     # timing bisect: omit segment matmuls
GATHER_QUEUES = 1        # rotate dma_gather queue_num over this many queues


# ----------------------------------------------------------------- host prep


def preprocess(edge_row, edge_col, edge_val):
    """Edge partitioning/sorting/padding with per-core dest->window
    rebalancing (greedy 2D bin-pack on lo/hi in-degree). Returns per-core
    data arrays plus a core-uniform tile structure."""
    edge_row = edge_row.astype(np.int64)
    edge_col = edge_col.astype(np.int64)

    # --- lo/hi in-degree per destination (lo = src owned by cores 0..3)
    src_core0 = edge_col // NLOC
    src_hi0 = src_core0 >= (NCORES // 2)
    lo_deg = np.zeros(N_NODES, np.int64)
    hi_deg = np.zeros(N_NODES, np.int64)
    np.add.at(lo_deg, edge_row, ~src_hi0)
    np.add.at(hi_deg, edge_row, src_hi0)

    # --- per-core greedy assignment of dests to windows (balance lo & hi)
    pos_of_node = np.zeros(N_NODES, np.int64)
    perms = []  # per core: padded_pos[d_local]
    for c in range(NCORES):
        ld = lo_deg[c * NLOC:(c + 1) * NLOC].astype(np.float64)
        hd = hi_deg[c * NLOC:(c + 1) * NLOC].astype(np.float64)
        order = np.argsort(-(ld + hd), kind="stable")
        # windows 0..N_BIG-1 get a 9-tile budget (1152); rest hard-capped at
        # 1024 so they stay 8 tiles. Every core's excess lands in the same
        # window indices, so the cross-core max stays tight.
        N_BIG = 5
        cap = np.full(W_WIN, 8.0 * P)
        cap[:N_BIG] = 9.0 * P
        loads_lo = np.zeros(W_WIN)
        loads_hi = np.zeros(W_WIN)
        counts = np.zeros(W_WIN, np.int64)
        wassign = np.zeros(NLOC, np.int64)
        for t, d in enumerate(order):
            ccap = min(P, t // W_WIN + 2)  # stay within 2 of even fill
            cost = np.maximum(loads_lo + ld[d], loads_hi + hd[d])
            infeas = ((counts >= ccap) | (loads_lo + ld[d] > cap)
                      | (loads_hi + hd[d] > cap))
            if infeas.all():
                infeas = counts >= ccap
            if infeas.all():
                infeas = counts >= P
            cost = np.where(infeas, 1e18, cost)
            w = int(np.argmin(cost))
            wassign[d] = w
            loads_lo[w] += ld[d]
            loads_hi[w] += hd[d]
            counts[w] += 1
        # slot within window
        slot_in_w = np.zeros(NLOC, np.int64)
        fill = np.zeros(W_WIN, np.int64)
        for d in range(NLOC):
            w = wassign[d]
            slot_in_w[d] = fill[w]
            fill[w] += 1
        p = wassign * P + slot_in_w
        perms.append(p)
        pos_of_node[c * NLOC:(c + 1) * NLOC] = c * NPAD + p

    core = edge_row // NLOC
    p_local = pos_of_node[edge_row] - core * NPAD
    win = p_local // P
    d8 = p_local % P
    srcg = pos_of_node[edge_col]
    is_hi = (srcg >= HALFROWS).astype(np.int64)

    lo_cnt = np.zeros((NCORES, W_WIN), np.int64)
    hi_cnt = np.zeros((NCORES, W_WIN), np.int64)
    np.add.at(lo_cnt, (core, win), 1 - is_hi)
    np.add.at(hi_cnt, (core, win), is_hi)

    lo_T = np.maximum(1, -(-lo_cnt.max(axis=0) // P))
    hi_T = -(-hi_cnt.max(axis=0) // P)
    win_T = lo_T + hi_T
    T_total = int(win_T.sum())

    # Paired-window stream: [wA_lo | wB_lo | wA_hi | wB_hi] so one gather
    # call covers both windows' lo (resp. hi) tiles -> fewer, bigger calls.
    lo_base = np.zeros(W_WIN, np.int64)
    hi_base = np.zeros(W_WIN, np.int64)
    pairs = [tuple(range(p, min(p + 2, W_WIN))) for p in range(0, W_WIN, 2)]
    gcalls = []   # per pair: [(tile_offset, n_tiles, is_hi), ...]
    spans = {}    # window -> [(tile_offset, n_tiles), ...] for matmuls
    t = 0
    for pr in pairs:
        t0 = t
        for w in pr:
            lo_base[w] = t * P
            spans[w] = [(t, int(lo_T[w]))]
            t += int(lo_T[w])
        calls = [(t0, t - t0, 0)]
        t1 = t
        for w in pr:
            hi_base[w] = t * P
            if hi_T[w] > 0:
                spans[w].append((t, int(hi_T[w])))
            t += int(hi_T[w])
        if t > t1:
            calls.append((t1, t - t1, 1))
        gcalls.append(calls)
    assert t == T_total
    runs = None  # superseded by pairs/gcalls/spans

    idx_all = np.zeros((NCORES, T_total * P), np.int16)
    oh_val = np.zeros((NCORES, T_total * P), np.float32)
    oh_d8 = np.zeros((NCORES, T_total * P), np.int64)

    order = np.lexsort((is_hi, win, core))
    e_core, e_win, e_hi = core[order], win[order], is_hi[order]
    e_srcg, e_d8, e_val = srcg[order], d8[order], edge_val[order]

    grp = (e_core * W_WIN + e_win) * 2 + e_hi
    cnt = np.zeros(NCORES * W_WIN * 2 + 1, np.int64)
    np.add.at(cnt, grp + 1, 1)
    starts = np.cumsum(cnt)[:-1]
    within = np.arange(len(order)) - starts[grp]

    slot = np.where(e_hi == 0, lo_base[e_win], hi_base[e_win]) + within
    idx_all[e_core, slot] = (e_srcg - e_hi * HALFROWS).astype(np.int16)
    oh_val[e_core, slot] = e_val
    oh_d8[e_core, slot] = e_d8

    return dict(idx_all=idx_all, oh_val=oh_val, oh_d8=oh_d8, pairs=pairs,
                gcalls=gcalls, spans=spans, win_T=win_T, T_total=T_total,
                perms=perms)


def build_in_maps(inputs, pp):
    raw_x = np.ascontiguousarray(inputs["raw_x"], dtype=np.float32)
    T = pp["T_total"]

    def wblocks(w, fout_pad):
        wp = np.zeros((w.shape[0], fout_pad), np.float32)
        wp[:, :w.shape[1]] = w
        kh = w.shape[0] // P
        return wp.reshape(kh, P, fout_pad)

    W012 = np.stack([
        np.stack([np.stack([inputs[k][a * P:(a + 1) * P, b * P:(b + 1) * P]
                            for b in range(2)]) for a in range(2)])
        for k in ("W_gc0", "W_gc1", "W_gc2")]).astype(np.float32)
    W3 = wblocks(np.asarray(inputs["W_gc3"], np.float32), 128)
    Wr0 = np.stack([np.stack([inputs["W_res0"][a * P:(a + 1) * P, b * P:(b + 1) * P]
                              for b in range(2)]) for a in range(2)]).astype(np.float32)
    WrL = wblocks(np.asarray(inputs["W_res_last"], np.float32), 64)
    ident = np.eye(P, dtype=np.float32)

    in_maps = []
    for c in range(NCORES):
        xT0 = np.zeros((2, P, NPAD), np.float32)
        xloc = raw_x[c * NLOC:(c + 1) * NLOC]          # [NLOC, 256]
        xT0[:, :, pp["perms"][c]] = xloc.T.reshape(2, P, NLOC)

        idx16 = np.zeros((16, T * 8), np.int16)
        i = np.arange(T * P)
        idx16[i % 16, i // 16] = pp["idx_all"][c]
        gidx = np.tile(idx16, (8, 1))                  # [128, T*8]

        import ml_dtypes
        oh = np.zeros((T * P, P), np.float32)
        oh[np.arange(T * P), pp["oh_d8"][c]] = pp["oh_val"][c]
        oh = np.ascontiguousarray(
            oh.reshape(T, P, P).transpose(1, 0, 2)).astype(ml_dtypes.bfloat16)

        in_maps.append(dict(xT0=xT0, gidx=gidx, oh=oh, W012=W012, W3=W3,
                            Wr0=Wr0, WrL=WrL, ident=ident))
    return in_maps


# ------------------------------------------------------------- device program


def build_program(pp):
    T = pp["T_total"]
    win_T = pp["win_T"]
    pairs, gcalls, spans = pp["pairs"], pp["gcalls"], pp["spans"]
    Tmax = max(sum(int(win_T[w]) for w in pr) for pr in pairs)
    NG = [(g * 512, min(512, NPAD - g * 512)) for g in range(-(-NPAD // 512))]

    nc = bacc.Bacc("TRN2", target_bir_lowering=False, debug=False,
                   num_devices=NCORES)

    xT0_d = nc.dram_tensor("xT0", [2, P, NPAD], F32, kind="ExternalInput")
    gidx_d = nc.dram_tensor("gidx", [P, T * 8], I16, kind="ExternalInput")
    oh_d = nc.dram_tensor("oh", [P, T, P], BF16, kind="ExternalInput")
    W012_d = nc.dram_tensor("W012", [3, 2, 2, P, P], F32, kind="ExternalInput")
    W3_d = nc.dram_tensor("W3", [2, P, 128], F32, kind="ExternalInput")
    Wr0_d = nc.dram_tensor("Wr0", [2, 2, P, P], F32, kind="ExternalInput")
    WrL_d = nc.dram_tensor("WrL", [2, P, 64], F32, kind="ExternalInput")
    ident_d = nc.dram_tensor("ident", [P, P], F32, kind="ExternalInput")
    out_d = nc.dram_tensor("out", [NPAD, C_OUT], F32, kind="ExternalOutput")

    res0_d = nc.dram_tensor("res0_d", [NPAD, HID], F32)
    rlast_d = nc.dram_tensor("rlast_d", [NPAD, 64], F32)
    ag_in = [nc.dram_tensor(f"ag_in{i}", [NPAD, HID], BF16) for i in range(2)]
    hfull = [nc.dram_tensor(f"hfull{i}", [NCORES * NPAD, HID], BF16,
                            addr_space="Shared") for i in range(2)]
    ag3_in = nc.dram_tensor("ag3_in", [NPAD, 128], BF16)
    h3full = nc.dram_tensor("h3full", [NCORES * NPAD, 128], BF16,
                            addr_space="Shared")

    with tile.TileContext(nc) as tc:
        with (
            tc.tile_pool(name="pers", bufs=1) as pers,
            tc.tile_pool(name="wbufp", bufs=3) as wbufp,
            tc.tile_pool(name="ohp", bufs=3) as ohp,
            tc.tile_pool(name="hTs", bufs=3) as hTsp,
            tc.tile_pool(name="hns", bufs=4) as hnsp,
            tc.tile_pool(name="zs", bufs=4) as zsp,
            tc.tile_pool(name="r0s", bufs=4) as r0sp,
            tc.tile_pool(name="sm", bufs=6) as smp,
            tc.tile_pool(name="ps_s", bufs=4, space="PSUM") as ps_s,
            tc.tile_pool(name="ps_d", bufs=2, space="PSUM") as ps_d,
            tc.tile_pool(name="ps_t", bufs=2, space="PSUM") as ps_t,
        ):
            # ---------------- persistent loads
            xT = pers.tile([P, 2, NPAD], F32, tag="xT")
            nc.sync.dma_start(out=xT[:, 0, :], in_=xT0_d.ap()[0])
            nc.sync.dma_start(out=xT[:, 1, :], in_=xT0_d.ap()[1])
            idxs = pers.tile([P, T * 8], I16, tag="gidx")
            nc.sync.dma_start(out=idxs[:], in_=gidx_d.ap())
            w012 = pers.tile([P, 12, P], F32, tag="w012")
            nc.sync.dma_start(
                out=w012[:],
                in_=W012_d.ap().rearrange("a b c p m -> p (a b c) m"))
            w3 = pers.tile([P, 2, 128], F32, tag="w3")
            nc.sync.dma_start(out=w3[:], in_=W3_d.ap().rearrange("a p m -> p a m"))
            wr0 = pers.tile([P, 4, P], F32, tag="wr0")
            nc.sync.dma_start(out=wr0[:],
                              in_=Wr0_d.ap().rearrange("a b p m -> p (a b) m"))
            wrl = pers.tile([P, 2, 64], F32, tag="wrl")
            nc.sync.dma_start(out=wrl[:], in_=WrL_d.ap().rearrange("a p m -> p a m"))
            ident = pers.tile([P, P], F32, tag="ident")
            nc.sync.dma_start(out=ident[:], in_=ident_d.ap())

            def w012_ap(li, kh, fh):
                return w012[:, li * 4 + kh * 2 + fh, :]

            # ------------- dense helper: h[:, fh*128:...] = x @ W  (+ hooks)
            def dense256(w_ap_fn, dest_dram, rl_hook=None, hn_dt=BF16,
                         hn_tag="hn"):
                """w_ap_fn(kh, fh) -> lhsT [128,128]. Writes node-major
                [NPAD, 256] to dest_dram. rl_hook(fh, g0, ng, hT_stage)."""
                for g0, ng in NG:
                    stages = []
                    for fh in range(2):
                        psd = ps_d.tile([P, 512], F32, space="PSUM", tag="dps")
                        for kh in range(2):
                            nc.tensor.matmul(
                                psd[:, :ng], lhsT=w_ap_fn(kh, fh),
                                rhs=xT[:, kh, g0:g0 + ng],
                                start=(kh == 0), stop=(kh == 1))
                        hTst = hTsp.tile([P, 512], F32, tag="hT")
                        nc.scalar.copy(out=hTst[:, :ng], in_=psd[:, :ng])
                        stages.append(hTst)
                        if rl_hook is not None:
                            rl_hook(fh, g0, ng, hTst)
                    for s in range(ng // P):
                        hn = hnsp.tile([P, HID], hn_dt, tag=hn_tag)
                        for fh in range(2):
                            pst = ps_t.tile([P, P], F32, space="PSUM", tag="tp")
                            nc.tensor.transpose(
                                out=pst[:],
                                in_=stages[fh][:, s * P:(s + 1) * P],
                                identity=ident[:])
                            nc.vector.tensor_copy(
                                out=hn[:, fh * P:(fh + 1) * P], in_=pst[:])
                        nc.sync.dma_start(
                            out=dest_dram.ap()[g0 + s * P:g0 + (s + 1) * P, :],
                            in_=hn[:])

            # ---------------- init: res0 (+ rlast) from raw_x
            rl_ps = {}

            def rl_hook(fh, g0, ng, hTst):
                if fh == 0:
                    rl_ps["t"] = ps_d.tile([P, 512], F32, space="PSUM",
                                           tag="dps", name="psr")
                psr = rl_ps["t"]
                nc.tensor.matmul(psr[:64, :ng], lhsT=wrl[:, fh, :],
                                 rhs=hTst[:, :ng],
                                 start=(fh == 0), stop=(fh == 1))
                if fh == 1:
                    rlT = hTsp.tile([P, 512], F32, tag="hT")
                    nc.scalar.copy(out=rlT[:64, :ng], in_=psr[:64, :ng])
                    for s in range(ng // P):
                        pst = ps_t.tile([P, P], F32, space="PSUM", tag="tp")
                        nc.tensor.transpose(out=pst[:, :64],
                                            in_=rlT[:64, s * P:(s + 1) * P],
                                            identity=ident[:64, :64])
                        rln = hnsp.tile([P, 64], F32, tag="rln")
                        nc.vector.tensor_copy(out=rln[:], in_=pst[:, :64])
                        nc.sync.dma_start(
                            out=rlast_d.ap()[g0 + s * P:g0 + (s + 1) * P, :],
                            in_=rln[:])

            dense256(lambda kh, fh: wr0[:, kh * 2 + fh, :], res0_d,
                     rl_hook=rl_hook, hn_dt=F32, hn_tag="hnf")

            # ---------------- spmm helper (paired windows)
            owner = {}
            for w, sp in spans.items():
                for (t0, nt) in sp:
                    for t in range(t0, t0 + nt):
                        owner[t] = w

            def spmm(hfull_t, elem, psw, evict_fn):
                for ip, pr in enumerate(pairs):
                    p0 = spans[pr[0]][0][0]
                    nTp = sum(int(win_T[w]) for w in pr)
                    ohb = ohp.tile([P, Tmax, P], BF16, tag="ohb")
                    nc.sync.dma_start(out=ohb[:, :nTp, :],
                                      in_=oh_d.ap()[:, p0:p0 + nTp, :])
                    wb = wbufp.tile([P, Tmax, elem], BF16, tag="wb")
                    if SKIP_GATHER:
                        nc.vector.memset(wb[:, 0, :], 0.0)
                    for (t0, nt, hi) in gcalls[ip]:
                        if SKIP_GATHER:
                            continue
                        src = (hfull_t.ap()[HALFROWS:, :] if hi
                               else hfull_t.ap()[:HALFROWS, :])
                        nc.gpsimd.dma_gather(
                            wb[:, t0 - p0:t0 - p0 + nt, :], src,
                            idxs[:, t0 * 8:(t0 + nt) * 8],
                            nt * P, nt * P, elem,
                            single_packet=GATHER_SINGLE_PACKET)
                    psl = {w: ps_s.tile([P, HID], F32, space="PSUM",
                                        tag="sps", name=f"pss_{ip}_{w}")
                           for w in pr}
                    cnt = {w: 0 for w in pr}
                    for t in range(p0, p0 + nTp):
                        w = owner[t]
                        nc.tensor.matmul(psl[w][:, :psw],
                                         lhsT=ohb[:, t - p0, :],
                                         rhs=wb[:, t - p0, :],
                                         start=(cnt[w] == 0),
                                         stop=(cnt[w] == int(win_T[w]) - 1))
                        cnt[w] += 1
                    for w in pr:
                        evict_fn(w, psl[w])

            # ---------------- GCN layers 0..2
            for li in range(3):
                dense256(lambda kh, fh, li=li: w012_ap(li, kh, fh), ag_in[li % 2])
                nc.gpsimd.collective_compute(
                    "AllGather", mybir.AluOpType.bypass,
                    replica_groups=[list(range(NCORES))],
                    ins=[ag_in[li % 2].ap()], outs=[hfull[li % 2].ap()])

                def evict_gc(w, pss):
                    r0 = r0sp.tile([P, HID], F32, tag="r0")
                    nc.sync.dma_start(out=r0[:],
                                      in_=res0_d.ap()[w * P:(w + 1) * P, :])
                    z = zsp.tile([P, HID], F32, tag="z")
                    nc.vector.tensor_add(out=z[:], in0=pss[:], in1=r0[:])
                    for fh in range(2):
                        pst = ps_t.tile([P, P], F32, space="PSUM", tag="tp")
                        nc.tensor.transpose(out=pst[:],
                                            in_=z[:, fh * P:(fh + 1) * P],
                                            identity=ident[:])
                        nc.scalar.activation(
                            out=xT[:, fh, w * P:(w + 1) * P], in_=pst[:],
                            func=mybir.ActivationFunctionType.Relu)

                spmm(hfull[li % 2], HID, HID, evict_gc)

            # ---------------- layer 3 dense (fout=64 incl. padding)
            for g0, ng in NG:
                psd = ps_d.tile([P, 512], F32, space="PSUM", tag="dps")
                for kh in range(2):
                    nc.tensor.matmul(psd[:, :ng], lhsT=w3[:, kh, :],
                                     rhs=xT[:, kh, g0:g0 + ng],
                                     start=(kh == 0), stop=(kh == 1))
                h3T = hTsp.tile([P, 512], F32, tag="hT")
                nc.scalar.copy(out=h3T[:, :ng], in_=psd[:, :ng])
                for s in range(ng // P):
                    pst = ps_t.tile([P, P], F32, space="PSUM", tag="tp")
                    nc.tensor.transpose(out=pst[:],
                                        in_=h3T[:, s * P:(s + 1) * P],
                                        identity=ident[:])
                    h3n = hnsp.tile([P, 128], BF16, tag="h3n")
                    nc.vector.tensor_copy(out=h3n[:], in_=pst[:])
                    nc.sync.dma_start(
                        out=ag3_in.ap()[g0 + s * P:g0 + (s + 1) * P, :],
                        in_=h3n[:])
            nc.gpsimd.collective_compute(
                "AllGather", mybir.AluOpType.bypass,
                replica_groups=[list(range(NCORES))],
                ins=[ag3_in.ap()], outs=[h3full.ap()])

            # ---------------- layer 3 spmm + rlast + log_softmax
            def evict_out(w, pss):
                rl = r0sp.tile([P, 64], F32, tag="rl")
                nc.sync.dma_start(out=rl[:],
                                  in_=rlast_d.ap()[w * P:(w + 1) * P, :])
                y = zsp.tile([P, 64], F32, tag="y")
                nc.vector.tensor_add(out=y[:], in0=pss[:, :64], in1=rl[:])
                m = smp.tile([P, 1], F32, tag="m")
                nc.vector.tensor_reduce(out=m[:], in_=y[:, :C_OUT],
                                        axis=mybir.AxisListType.X,
                                        op=mybir.AluOpType.max)
                tt = smp.tile([P, C_OUT], F32, tag="tt")
                nc.vector.tensor_scalar(out=tt[:], in0=y[:, :C_OUT],
                                        scalar1=m[:], scalar2=None,
                                        op0=mybir.AluOpType.subtract)
                e = smp.tile([P, C_OUT], F32, tag="e")
                ssum = smp.tile([P, 1], F32, tag="ss")
                nc.scalar.activation(out=e[:], in_=tt[:],
                                     func=mybir.ActivationFunctionType.Exp,
                                     accum_out=ssum[:])
                lg = smp.tile([P, 1], F32, tag="lg")
                nc.scalar.activation(out=lg[:], in_=ssum[:],
                                     func=mybir.ActivationFunctionType.Ln)
                o = smp.tile([P, C_OUT], F32, tag="o")
                nc.vector.tensor_scalar(out=o[:], in0=tt[:], scalar1=lg[:],
                                        scalar2=None,
                                        op0=mybir.AluOpType.subtract)
                nc.sync.dma_start(out=out_d.ap()[w * P:(w + 1) * P, :],
                                  in_=o[:])

            spmm(h3full, 128, 128, evict_out)

    nc.compile()
    return nc


# ------------------------------------------------------------ timed benchmark


def bench(nc, in_maps, iters=8):
    """Times repeated on-device executions with device-resident inputs
    (replicates bass2jax.run_bass_via_pjrt's multi-core path). Returns
    (best_seconds, per_iter_seconds)."""
    import time

    import jax
    from jax.experimental.shard_map import shard_map
    from jax.sharding import Mesh, NamedSharding, PartitionSpec

    from concourse import bass2jax, mybir as mb

    bass2jax.install_neuronx_cc_hook()

    partition_name = (nc.partition_id_tensor.name
                      if nc.partition_id_tensor else None)
    in_names, out_names, out_avals, zero_outs = [], [], [], []
    for alloc in nc.m.functions[0].allocations:
        if not isinstance(alloc, mb.MemoryLocationSet):
            continue
        name = alloc.memorylocations[0].name
        if alloc.kind == "ExternalInput":
            if name != partition_name:
                in_names.append(name)
        elif alloc.kind == "ExternalOutput":
            out_names.append(name)
            shape = tuple(alloc.tensor_shape)
            dtype = mb.dt.np(alloc.dtype)
            out_avals.append(jax.core.ShapedArray(shape, dtype))
            zero_outs.append(np.zeros(shape, dtype))
    n_params = len(in_names)
    n_outs = len(out_avals)
    all_names = in_names + out_names
    if partition_name is not None:
        all_names = all_names + [partition_name]

    def _body(*args):
        operands = list(args)
        if partition_name is not None:
            operands.append(bass2jax.partition_id_tensor())
        outs = bass2jax._bass_exec_p.bind(
            *operands, out_avals=tuple(out_avals), in_names=tuple(all_names),
            out_names=tuple(out_names), lowering_input_output_aliases=(),
            sim_require_finite=True, sim_require_nnan=True, nc=nc)
        return tuple(outs)

    devices = jax.devices()[:NCORES]
    mesh = Mesh(np.asarray(devices), ("core",))
    in_specs = (PartitionSpec("core"),) * (n_params + n_outs)
    out_specs = (PartitionSpec("core"),) * n_outs
    donate = tuple(range(n_params, n_params + n_outs))
    del donate
    sharded = jax.jit(shard_map(_body, mesh=mesh, in_specs=in_specs,
                                out_specs=out_specs, check_rep=False),
                      keep_unused=True)

    sh = NamedSharding(mesh, PartitionSpec("core"))
    dev_in = [
        jax.device_put(
            np.concatenate([np.asarray(in_maps[c][n]) for c in range(NCORES)],
                           axis=0), sh)
        for n in in_names]
    zglobal = [np.zeros((NCORES * z.shape[0], *z.shape[1:]), z.dtype)
               for z in zero_outs]

    dz = [jax.device_put(z, sh) for z in zglobal]
    for d in dz:
        d.block_until_ready()

    def run_batch(n):
        """Queue n executions without intermediate sync; return elapsed."""
        t0 = time.perf_counter()
        outs = None
        for _ in range(n):
            outs = sharded(*dev_in, *dz)
        for o in outs:
            o.block_until_ready()
        return time.perf_counter() - t0

    run_batch(1)  # warmup
    n_hi = 41
    lo = min(run_batch(1) for _ in range(iters))
    hi = min(run_batch(n_hi) for _ in range(max(2, iters // 2)))
    per_exec = (hi - lo) / (n_hi - 1)
    return per_exec, (lo, hi)


# ---------------------------------------------------------------- entry point

_CACHE = {}


def _run(inputs, trace=False, trace_kwargs=None):
    pp = preprocess(np.asarray(inputs["edge_row"]),
                    np.asarray(inputs["edge_col"]),
                    np.asarray(inputs["edge_val"], dtype=np.float32))
    in_maps = build_in_maps(inputs, pp)
    key = (pp["T_total"], tuple(tuple(c) for cs in pp["gcalls"] for c in cs))
    if key not in _CACHE:
        _CACHE.clear()
        _CACHE[key] = build_program(pp)
    nc = _CACHE[key]
    res = run_bass_kernel_spmd(nc, in_maps, list(range(NCORES)), trace=trace,
                               **(trace_kwargs or {}))
    outs = [res.results[c]["out"][pp["perms"][c]] for c in range(NCORES)]
    full = np.concatenate(outs, axis=0).astype(np.float32)
    return full, res


def kernel(**inputs):
    out, _ = _run(inputs)
    return out
